# revision 1
# baseline (speedup 1.0000x reference)
"""TransformerConv MixerBlock (x + TransformerConv(x, edge_index)) on 8 trn2 NeuronCores.

Strategy: permute+bin-pack nodes into 128-node tiles balanced by in-degree
(49 tiles/core). Each core builds the full K/V table (fp16, 512B rows) from x,
then processes its own destination tiles: one indirect-DMA gather of the
incident edges' kv rows per tile, one-hot matmuls to recover q per edge and to
scatter-accumulate exp(alpha)*[v|1] into PSUM, then normalize + skip + residual.
"""
import sys, os, types, math, heapq
sys.path.insert(0, '/opt/trn_rl_repo')
import numpy as np

P = 128
D = 128
H = 4
DH = 32
NCORES = 8

_prog_cache = {}


def _ensure_hooks():
    """Best-effort shim of antenv.axon_hooks so trace=True profiling works."""
    try:
        import antenv
        if 'antenv.axon_hooks' not in sys.modules:
            mod = types.ModuleType('antenv.axon_hooks')
            state = {'hook': None}
            mod.set_axon_ntff_profile_hook = lambda h: state.__setitem__('hook', h)
            mod.get_axon_ntff_profile_hook = lambda: state['hook']
            sys.modules['antenv.axon_hooks'] = mod
            antenv.axon_hooks = mod
            from trn_agent_boot.trn_boot import _ntff_profile_via_ctypes
            hook = _ntff_profile_via_ctypes('/opt/axon/libaxon_pjrt.so')
            if hook is not None:
                mod.set_axon_ntff_profile_hook(hook)
    except Exception:
        pass
    try:
        import concourse.bass_utils as bass_utils
        bass_utils.upload_artifacts = lambda tmpdir: tmpdir
    except Exception:
        pass


def _prep(x, edge_index, Wq, bq, Wk, bk, Wv, bv, Wskip, bskip):
    N = x.shape[0]
    E = edge_index.shape[1]
    TPC = (N + NCORES * P - 1) // (NCORES * P)
    NT = NCORES * TPC

    src = np.asarray(edge_index[0], dtype=np.int64)
    dst = np.asarray(edge_index[1], dtype=np.int64)
    deg = np.bincount(dst, minlength=N)

    # --- bin-pack nodes into NT tiles of <=P nodes, balancing degree sums ---
    order = np.argsort(-deg, kind='stable')
    heap = [(0, t) for t in range(NT)]
    heapq.heapify(heap)
    counts = np.zeros(NT, dtype=np.int64)
    tile_deg = np.zeros(NT, dtype=np.int64)
    node_slot = np.empty(N, dtype=np.int64)
    for n in order:
        while True:
            dsum, t = heapq.heappop(heap)
            if counts[t] < P:
                break
        node_slot[n] = t * P + counts[t]
        counts[t] += 1
        tile_deg[t] += deg[n]
        if counts[t] < P:
            heapq.heappush(heap, (dsum + int(deg[n]), t))
    K = max(1, int((tile_deg.max() + P - 1) // P))

    # --- permuted node features ---
    x_perm = np.zeros((NT * P, D), dtype=np.float16)
    x_perm[node_slot] = np.asarray(x, dtype=np.float16)

    # --- per-tile edge lists sorted by src slot, padded to K*P ---
    src_slot = node_slot[src]
    dst_slot = node_slot[dst]
    et = dst_slot // P            # destination tile of each edge
    key = et * (1 << 32) + src_slot
    eorder = np.argsort(key, kind='stable')
    et_s = et[eorder]
    src_s = src_slot[eorder].astype(np.int32)
    dloc_s = (dst_slot[eorder] - et_s * P).astype(np.int64)

    ecnt = np.bincount(et, minlength=NT)
    eoff = np.zeros(NT + 1, dtype=np.int64)
    np.cumsum(ecnt, out=eoff[1:])
    pos = np.arange(E) - eoff[et_s]          # position within its tile
    flat = et_s * (K * P) + pos

    src_pad = np.zeros(NT * K * P, dtype=np.int32)
    dloc_pad = np.full(NT * K * P, 255, dtype=np.int64)
    src_pad[flat] = src_s
    dloc_pad[flat] = dloc_s

    # gather layout [NT, P, K]: slot (j*P + p) -> [p, j]
    src3 = src_pad.reshape(NT, K, P)
    # chunk j of any tile only references kv rows < hb[j] (edges sorted by src)
    hb = src3.max(axis=(0, 2)).astype(np.int64) + 1
    NBP = 4 * P
    hb = np.minimum(((hb + NBP - 1) // NBP) * NBP, NT * P)
    hb = np.maximum.accumulate(hb)
    src_g = src3.transpose(0, 2, 1).copy()
    dloc_g = dloc_pad.reshape(NT, K, P).transpose(0, 2, 1).astype(np.float32).copy()
    # one-hot [NT, P(r), K*P(c)] where col c = j*P + e
    oh = (dloc_pad.reshape(NT, 1, K * P) == np.arange(P).reshape(1, P, 1))
    oh = oh.astype(np.float16)

    s = 1.0 / math.sqrt(DH)
    wkT = np.asarray(Wk, dtype=np.float32).T.astype(np.float16).copy()
    wvT = np.asarray(Wv, dtype=np.float32).T.astype(np.float16).copy()
    wqT = (np.asarray(Wq, dtype=np.float32).T * s).astype(np.float16).copy()
    wsT = np.asarray(Wskip, dtype=np.float32).T.astype(np.float16).copy()
    for b in (bq, bk, bv, bskip):
        assert np.abs(np.asarray(b)).max() == 0.0, "nonzero biases not supported"
    iota = np.tile(np.arange(P, dtype=np.float16).reshape(1, P), (P, 1)).copy()

    in_maps = []
    for c in range(NCORES):
        t0, t1 = c * TPC, (c + 1) * TPC
        in_maps.append({
            "x_perm": x_perm,
            "x_loc": x_perm[t0 * P:t1 * P].copy(),
            "wkT": wkT, "wvT": wvT, "wqT": wqT, "wsT": wsT, "iota": iota,
            "src_idx": src_g[t0:t1].reshape(TPC * P, K).copy(),
            "dloc": dloc_g[t0:t1].reshape(TPC * P, K).copy(),
            "oh": oh[t0:t1].reshape(TPC * P, K * P).copy(),
        })
    return dict(N=N, E=E, TPC=TPC, NT=NT, K=K, node_slot=node_slot,
                hb=tuple(int(v) for v in hb), in_maps=in_maps)


def _build(TPC, NT, K, HB):
    import concourse.bass as bass
    import concourse.bacc as bacc
    import concourse.mybir as mybir
    import concourse.tile as tile

    f16 = mybir.dt.float16
    f32 = mybir.dt.float32
    i32 = mybir.dt.int32
    MUL = mybir.AluOpType.mult
    ADD = mybir.AluOpType.add
    ISEQ = mybir.AluOpType.is_equal
    EXP = mybir.ActivationFunctionType.Exp
    COPY = mybir.ActivationFunctionType.Copy

    nc = bacc.Bacc("TRN2", target_bir_lowering=False, debug=False)
    x_perm = nc.dram_tensor("x_perm", [NT * P, D], f16, kind="ExternalInput")
    x_loc = nc.dram_tensor("x_loc", [TPC * P, D], f16, kind="ExternalInput")
    wkT = nc.dram_tensor("wkT", [D, D], f16, kind="ExternalInput")
    wvT = nc.dram_tensor("wvT", [D, D], f16, kind="ExternalInput")
    wqT = nc.dram_tensor("wqT", [D, D], f16, kind="ExternalInput")
    wsT = nc.dram_tensor("wsT", [D, D], f16, kind="ExternalInput")
    iota = nc.dram_tensor("iota", [P, P], f16, kind="ExternalInput")
    src_idx = nc.dram_tensor("src_idx", [TPC * P, K], i32, kind="ExternalInput")
    dloc = nc.dram_tensor("dloc", [TPC * P, K], f32, kind="ExternalInput")
    oh_in = nc.dram_tensor("oh", [TPC * P, K * P], f16, kind="ExternalInput")
    out_t = nc.dram_tensor("out", [TPC * P, D], f32, kind="ExternalOutput")

    kv_table = nc.dram_tensor("kv_table", [NT * P, 256], f16)

    NB = 4
    assert NT % NB == 0
    groups = [(g * 8, min(8, K - g * 8)) for g in range((K + 7) // 8)]

    with tile.TileContext(nc) as tc:
        with (
            tc.tile_pool(name="const", bufs=1) as cp,
            tc.tile_pool(name="sbuf", bufs=4) as sb,
            tc.tile_pool(name="big", bufs=4) as bigp,
            tc.tile_pool(name="psA", bufs=2, space="PSUM") as psA,
            tc.tile_pool(name="psB", bufs=2, space="PSUM") as psB,
        ):
            wkv_sb = cp.tile([D, 256], f16, tag="wkv")
            wqs_sb = cp.tile([D, 256], f16, tag="wqs")
            iota_sb = cp.tile([P, P], f16, tag="iota")
            q_loc = cp.tile([P, TPC * D], f16, tag="qloc")
            s_loc = cp.tile([P, TPC * D], f16, tag="sloc")
            nc.sync.dma_start(out=wkv_sb[:, 0:128], in_=wkT[:])
            nc.sync.dma_start(out=wkv_sb[:, 128:256], in_=wvT[:])
            nc.sync.dma_start(out=wqs_sb[:, 0:128], in_=wqT[:])
            nc.sync.dma_start(out=wqs_sb[:, 128:256], in_=wsT[:])
            nc.sync.dma_start(out=iota_sb[:], in_=iota[:])

            # ---------------- node phase: full kv table ----------------
            for it in range(NT // NB):
                t0 = it * NB
                xT = sb.tile([P, NB * P], f16, tag="xT")
                nc.sync.dma_start(
                    out=xT[:], in_=x_perm[t0 * P:(t0 + NB) * P, :], transpose=True)
                pkv = psA.tile([P, NB * 256], f32, tag="pbig")
                for b in range(NB):
                    nc.tensor.matmul(pkv[:, b * 256:(b + 1) * 256],
                                     lhsT=xT[:, b * P:(b + 1) * P], rhs=wkv_sb[:],
                                     start=True, stop=True)
                kvt = sb.tile([P, NB * 256], f16, tag="kvt")
                nc.scalar.activation(out=kvt[:], in_=pkv[:], func=COPY)
                nc.sync.dma_start(
                    out=kv_table[t0 * P:(t0 + NB) * P, :].rearrange(
                        "(b p) c -> p b c", p=P),
                    in_=kvt[:].rearrange("p (b c) -> p b c", c=256))

            # ---------------- local phase: q and skip ----------------
            u = 0
            while u < TPC:
                lb = min(NB, TPC - u)
                xTl = sb.tile([P, NB * P], f16, tag="xT")
                nc.sync.dma_start(
                    out=xTl[:, :lb * P], in_=x_loc[u * P:(u + lb) * P, :],
                    transpose=True)
                pq = psA.tile([P, NB * 256], f32, tag="pbig")
                for b in range(lb):
                    nc.tensor.matmul(pq[:, b * 256:(b + 1) * 256],
                                     lhsT=xTl[:, b * P:(b + 1) * P], rhs=wqs_sb[:],
                                     start=True, stop=True)
                nc.scalar.activation(
                    out=q_loc[:, u * D:(u + lb) * D].rearrange(
                        "p (b c) -> p b c", c=P),
                    in_=pq[:, :lb * 256].rearrange(
                        "p (b c) -> p b c", c=256)[:, :, 0:128], func=COPY)
                xl = sb.tile([P, NB, P], f16, tag="xl")
                nc.sync.dma_start(
                    out=xl[:, :lb, :],
                    in_=x_loc[u * P:(u + lb) * P, :].rearrange(
                        "(b p) c -> p b c", p=P))
                nc.vector.tensor_tensor(
                    out=s_loc[:, u * D:(u + lb) * D].rearrange(
                        "p (b c) -> p b c", c=P),
                    in0=pq[:, :lb * 256].rearrange(
                        "p (b c) -> p b c", c=256)[:, :, 128:256],
                    in1=xl[:, :lb, :], op=ADD)
                u += lb

            # ---------------- edge phase ----------------
            for u in range(TPC):
                idx = sb.tile([P, K], i32, tag="idx")
                nc.sync.dma_start(out=idx[:], in_=src_idx[u * P:(u + 1) * P, :])
                dl = sb.tile([P, K], f32, tag="dl")
                nc.sync.dma_start(out=dl[:], in_=dloc[u * P:(u + 1) * P, :])
                kv_g = bigp.tile([P, K, 256], f16, tag="kvg")
                for j in range(K):
                    nc.gpsimd.indirect_dma_start(
                        out=kv_g[:, j, :], out_offset=None,
                        in_=kv_table[0:HB[j], :],
                        in_offset=bass.IndirectOffsetOnAxis(
                            ap=idx[:, j:j + 1], axis=0))
                oh = bigp.tile([P, K * P], f16, tag="oh")
                nc.sync.dma_start(out=oh[:], in_=oh_in[u * P:(u + 1) * P, :])

                psS = psB.tile([P, 132], f32, tag="acc")
                for (j0, gsz) in groups:
                    pqe = psA.tile([P, NB * 256], f32, tag="pbig")
                    for jj in range(gsz):
                        j = j0 + jj
                        nc.tensor.matmul(
                            pqe[:, jj * P:(jj + 1) * P],
                            lhsT=oh[:, j * P:(j + 1) * P],
                            rhs=q_loc[:, u * D:(u + 1) * D],
                            start=True, stop=True)
                    ohT = sb.tile([P, 8, P], f16, tag="ohT")
                    for jj in range(gsz):
                        j = j0 + jj
                        nc.vector.tensor_scalar(
                            out=ohT[:, jj, :], in0=iota_sb[:],
                            scalar1=dl[:, j:j + 1], scalar2=None, op0=ISEQ)
                    qk = sb.tile([P, 8, P], f16, tag="qk")
                    nc.vector.tensor_tensor(
                        out=qk[:, :gsz, :],
                        in0=pqe[:, :gsz * P].rearrange("p (a c) -> p a c", c=P),
                        in1=kv_g[:, j0:j0 + gsz, 0:128], op=MUL)
                    # reduce 32 -> 1 within each head, via add tree (2x mode)
                    t16 = sb.tile([P, 8 * H, 16], f16, tag="t16")
                    qkv = qk[:, :gsz, :].rearrange("p a (h e) -> p (a h) e", e=DH)
                    nc.vector.tensor_tensor(out=t16[:, :gsz * H, :],
                                            in0=qkv[:, :, 0:16],
                                            in1=qkv[:, :, 16:32], op=ADD)
                    t8 = sb.tile([P, 8 * H, 8], f16, tag="t8")
                    nc.vector.tensor_tensor(out=t8[:, :gsz * H, :],
                                            in0=t16[:, :gsz * H, 0:8],
                                            in1=t16[:, :gsz * H, 8:16], op=ADD)
                    t4 = sb.tile([P, 8 * H, 4], f16, tag="t4")
                    nc.vector.tensor_tensor(out=t4[:, :gsz * H, :],
                                            in0=t8[:, :gsz * H, 0:4],
                                            in1=t8[:, :gsz * H, 4:8], op=ADD)
                    t2 = sb.tile([P, 8 * H, 2], f16, tag="t2")
                    nc.vector.tensor_tensor(out=t2[:, :gsz * H, :],
                                            in0=t4[:, :gsz * H, 0:2],
                                            in1=t4[:, :gsz * H, 2:4], op=ADD)
                    alpha = sb.tile([P, 8 * H, 1], f16, tag="alpha")
                    nc.vector.tensor_tensor(out=alpha[:, :gsz * H, :],
                                            in0=t2[:, :gsz * H, 0:1],
                                            in1=t2[:, :gsz * H, 1:2], op=ADD)
                    X = sb.tile([P, 8, 132], f16, tag="X")
                    nc.scalar.activation(
                        out=X[:, :gsz, 128:132],
                        in_=alpha[:, :gsz * H, 0:1].rearrange(
                            "p (a h) e -> p a (h e)", h=H),
                        func=EXP)
                    nc.vector.tensor_tensor(
                        out=X[:, :gsz, 0:128].rearrange("p a (h e) -> p a h e", e=DH),
                        in0=kv_g[:, j0:j0 + gsz, 128:256].rearrange(
                            "p a (h e) -> p a h e", e=DH),
                        in1=X[:, :gsz, 128:132, None].to_broadcast([P, gsz, H, DH]),
                        op=MUL)
                    for jj in range(gsz):
                        j = j0 + jj
                        nc.tensor.matmul(
                            psS[:, 0:132], lhsT=ohT[:, jj, :], rhs=X[:, jj, 0:132],
                            start=(j == 0), stop=(j == K - 1))
                dn = sb.tile([P, H], f32, tag="dn")
                nc.vector.tensor_scalar(out=dn[:], in0=psS[:, 128:132],
                                        scalar1=1e-16, scalar2=None, op0=ADD)
                rc = sb.tile([P, H], f32, tag="rc")
                nc.vector.reciprocal(out=rc[:], in_=dn[:])
                ot = sb.tile([P, D], f32, tag="ot")
                nc.vector.tensor_tensor(
                    out=ot[:].rearrange("p (h e) -> p h e", e=DH),
                    in0=psS[:, 0:128].rearrange("p (h e) -> p h e", e=DH),
                    in1=rc[:, :, None].to_broadcast([P, H, DH]), op=MUL)
                of = sb.tile([P, D], f32, tag="of")
                nc.vector.tensor_tensor(
                    out=of[:], in0=ot[:], in1=s_loc[:, u * D:(u + 1) * D], op=ADD)
                nc.sync.dma_start(out=out_t[u * P:(u + 1) * P, :], in_=of[:])

    nc.finalize()
    return nc


def _run(inputs, trace=False):
    _ensure_hooks()
    from concourse.bass_utils import run_bass_kernel_spmd

    meta = _prep(**inputs)
    key = (meta['TPC'], meta['NT'], meta['K'], meta['hb'])
    if key not in _prog_cache:
        _prog_cache[key] = _build(*key)
    nc = _prog_cache[key]
    res = run_bass_kernel_spmd(nc, meta['in_maps'],
                               core_ids=list(range(NCORES)), trace=trace)
    outs = [res.results[c]["out"] for c in range(NCORES)]
    out_perm = np.concatenate(outs, axis=0)
    out = out_perm[meta['node_slot']].astype(np.float32)
    return out, res


def kernel(**inputs) -> np.ndarray:
    out, _ = _run(inputs, trace=False)
    return out



# revision 21
# speedup vs baseline: 1.5037x; 1.5037x over previous
"""TransformerConv MixerBlock (x + TransformerConv(x, edge_index)) on 8 trn2 NeuronCores.

Strategy: permute+bin-pack nodes into 128-node tiles balanced by in-degree
(49 tiles/core). Each core builds the full K/V table (fp16, 512B rows) from x,
then processes its own destination tiles: per tile, TWO dma_gather calls (one
per table half, int16 idx limit) fetch all incident edges' kv rows in bulk,
then one-hot matmuls recover q per edge and scatter-accumulate
exp(alpha)*[v|1] into PSUM, then normalize + skip + residual.

v2 vs v1: indirect_dma_start (K per tile, ~1.4us Q7 descriptor-gen each,
GpSimd-engine-bound) replaced by 2 bulk dma_gather per tile; one-hot ohT built
with a single ISEQ tensor_tensor per 8-chunk group (iota blocks + host-biased
dloc); add-tree replaced by tensor_reduce; exp+head-broadcast moved to the
scalar engine so the v-multiply runs in DVE 2x mode.
"""
import sys, os, types, math, heapq
sys.path.insert(0, '/opt/trn_rl_repo')
import numpy as np

P = 128
D = 128
H = 4
DH = 32
NCORES = 8

# SAFE_COMPUTE=True keeps dma_gather but uses the v1-proven per-chunk compute
# (tensor_scalar ISEQ + add-tree + broadcast X-multiply) for HW bisection.
SAFE_COMPUTE = False

_prog_cache = {}


def _ensure_hooks():
    """Best-effort shim of antenv.axon_hooks so trace=True profiling works."""
    try:
        import antenv
        if 'antenv.axon_hooks' not in sys.modules:
            mod = types.ModuleType('antenv.axon_hooks')
            state = {'hook': None}
            mod.set_axon_ntff_profile_hook = lambda h: state.__setitem__('hook', h)
            mod.get_axon_ntff_profile_hook = lambda: state['hook']
            sys.modules['antenv.axon_hooks'] = mod
            antenv.axon_hooks = mod
            from trn_agent_boot.trn_boot import _ntff_profile_via_ctypes
            hook = _ntff_profile_via_ctypes('/opt/axon/libaxon_pjrt.so')
            if hook is not None:
                mod.set_axon_ntff_profile_hook(hook)
    except Exception:
        pass
    try:
        import concourse.bass_utils as bass_utils
        bass_utils.upload_artifacts = lambda tmpdir: tmpdir
    except Exception:
        pass


def _wrap_idx(vals, K):
    """Pack K*128 int16 indices into the dma_gather [128, K*8] layout:
    index i lives at [i%16, i//16], replicated across the 8 groups of 16
    partitions. Unused slots stay 0 (a valid row; its one-hot column is 0)."""
    arr = np.zeros(K * P, dtype=np.int16)
    arr[:len(vals)] = vals
    w = arr.reshape(K * 8, 16).T          # [16, K*8]
    return np.tile(w, (8, 1)).copy()      # [128, K*8]


def _prep(x, edge_index, Wq, bq, Wk, bk, Wv, bv, Wskip, bskip):
    N = x.shape[0]
    E = edge_index.shape[1]
    TPC = (N + NCORES * P - 1) // (NCORES * P)
    NT = NCORES * TPC
    HALF = (NT * P) // 2

    src = np.asarray(edge_index[0], dtype=np.int64)
    dst = np.asarray(edge_index[1], dtype=np.int64)
    deg = np.bincount(dst, minlength=N)

    # --- bin-pack nodes into NT tiles of <=P nodes, balancing degree sums ---
    order = np.argsort(-deg, kind='stable')
    heap = [(0, t) for t in range(NT)]
    heapq.heapify(heap)
    counts = np.zeros(NT, dtype=np.int64)
    tile_deg = np.zeros(NT, dtype=np.int64)
    node_slot = np.empty(N, dtype=np.int64)
    for n in order:
        while True:
            dsum, t = heapq.heappop(heap)
            if counts[t] < P:
                break
        node_slot[n] = t * P + counts[t]
        counts[t] += 1
        tile_deg[t] += deg[n]
        if counts[t] < P:
            heapq.heappush(heap, (dsum + int(deg[n]), t))

    # --- permuted node features (plus transposed copy: avoids DMA transpose) ---
    x_perm = np.zeros((NT * P, D), dtype=np.float16)
    x_perm[node_slot] = np.asarray(x, dtype=np.float16)
    x_permT = x_perm.T.copy()

    # --- per-tile edge lists sorted by src slot ---
    src_slot = node_slot[src]
    dst_slot = node_slot[dst]
    et = dst_slot // P            # destination tile of each edge
    key = et * (1 << 32) + src_slot
    eorder = np.argsort(key, kind='stable')
    et_s = et[eorder]
    src_s = src_slot[eorder]
    dloc_s = (dst_slot[eorder] - et_s * P)

    ecnt = np.bincount(et, minlength=NT)
    eoff = np.zeros(NT + 1, dtype=np.int64)
    np.cumsum(ecnt, out=eoff[1:])

    # per-tile half split (edges are sorted by src within each tile);
    # uniform KA/KB across tiles so one SPMD program fits every core
    splits = []
    KA = KB = 0
    for t in range(NT):
        s, e = eoff[t], eoff[t + 1]
        nlo = int(np.searchsorted(src_s[s:e], HALF, side='left'))
        nup = int(e - s) - nlo
        splits.append(nlo)
        KA = max(KA, (nlo + P - 1) // P)
        KB = max(KB, (nup + P - 1) // P)
    KU = KA + KB

    # --- per-tile padded idx / dlw / oh arrays ---
    idx_all = np.zeros((NT, P, KU * 8), dtype=np.int16)
    dlw_all = np.zeros((NT, P, KU), dtype=np.float16)
    dloc_all = np.zeros((NT, P, KU), dtype=np.float32)
    oh_all = np.zeros((NT, P, KU * P), dtype=np.float16)
    arange_kt = 256 * (np.arange(KU) % 8).reshape(1, KU)
    for t in range(NT):
        s, e = eoff[t], eoff[t + 1]
        nlo = splits[t]
        slo, dlo = src_s[s:s + nlo], dloc_s[s:s + nlo]
        sup, dup = src_s[s + nlo:e] - HALF, dloc_s[s + nlo:e]
        idx_all[t, :, :KA * 8] = _wrap_idx(slo, KA)
        idx_all[t, :, KA * 8:] = _wrap_idx(sup, KB)
        # edge i of a half -> chunk i//128, part i%128; upper chunks at KA.
        dl_pad = np.full(KU * P, 255, dtype=np.int64)
        dl_pad[:nlo] = dlo
        dl_pad[KA * P:KA * P + len(dup)] = dup
        # one-hot [P(row n), KU*P(col j*128+e)]
        oh_all[t] = (dl_pad.reshape(1, KU * P)
                     == np.arange(P).reshape(P, 1)).astype(np.float16)
        # dlw[part, chunk] = dloc + 256*(chunk%8)
        dlw_all[t] = (dl_pad.reshape(KU, P).T + arange_kt).astype(np.float16)
        dloc_all[t] = dl_pad.reshape(KU, P).T.astype(np.float32)

    s = 1.0 / math.sqrt(DH)
    wkT = np.asarray(Wk, dtype=np.float32).T.astype(np.float16).copy()
    wvT = np.asarray(Wv, dtype=np.float32).T.astype(np.float16).copy()
    wqT = (np.asarray(Wq, dtype=np.float32).T * s).astype(np.float16).copy()
    wsT = np.asarray(Wskip, dtype=np.float32).T.astype(np.float16).copy()
    for b in (bq, bk, bv, bskip):
        assert np.abs(np.asarray(b)).max() == 0.0, "nonzero biases not supported"
    # iota blocks: iota2048[p, a*128+c] = a*256 + c  (for ISEQ ohT build)
    io = (256 * np.arange(8).reshape(8, 1) + np.arange(P).reshape(1, P))
    iota2048 = np.tile(io.reshape(1, 8 * P), (P, 1)).astype(np.float16).copy()
    iota_pp = np.tile(np.arange(P, dtype=np.float16).reshape(1, P), (P, 1)).copy()

    in_maps = []
    for c in range(NCORES):
        t0, t1 = c * TPC, (c + 1) * TPC
        in_maps.append({
            "x_permT": x_permT,
            "x_loc": x_perm[t0 * P:t1 * P].copy(),
            "x_locT": x_permT[:, t0 * P:t1 * P].copy(),
            "wkT": wkT, "wvT": wvT, "wqT": wqT, "wsT": wsT,
            "iota2048": iota2048,
            "iota": iota_pp,
            "idx_in": idx_all[t0:t1].copy(),
            "dlw_in": dlw_all[t0:t1].reshape(TPC * P, KU).copy(),
            "dloc": dloc_all[t0:t1].reshape(TPC * P, KU).copy(),
            "oh": oh_all[t0:t1].reshape(TPC * P, KU * P).copy(),
        })
    return dict(N=N, E=E, TPC=TPC, NT=NT, KA=KA, KB=KB, HALF=HALF,
                node_slot=node_slot, in_maps=in_maps)


def _build(TPC, NT, KA, KB, HALF):
    import concourse.bass as bass
    import concourse.bacc as bacc
    import concourse.mybir as mybir
    import concourse.tile as tile

    f16 = mybir.dt.float16
    f32 = mybir.dt.float32
    i16 = mybir.dt.int16
    MUL = mybir.AluOpType.mult
    ADD = mybir.AluOpType.add
    ISEQ = mybir.AluOpType.is_equal
    EXP = mybir.ActivationFunctionType.Exp
    COPY = mybir.ActivationFunctionType.Copy
    AXX = mybir.AxisListType.X

    KU = KA + KB

    nc = bacc.Bacc("TRN2", target_bir_lowering=False, debug=False)
    x_permT = nc.dram_tensor("x_permT", [D, NT * P], f16, kind="ExternalInput")
    x_loc = nc.dram_tensor("x_loc", [TPC * P, D], f16, kind="ExternalInput")
    x_locT = nc.dram_tensor("x_locT", [D, TPC * P], f16, kind="ExternalInput")
    wkT = nc.dram_tensor("wkT", [D, D], f16, kind="ExternalInput")
    wvT = nc.dram_tensor("wvT", [D, D], f16, kind="ExternalInput")
    wqT = nc.dram_tensor("wqT", [D, D], f16, kind="ExternalInput")
    wsT = nc.dram_tensor("wsT", [D, D], f16, kind="ExternalInput")
    if SAFE_COMPUTE:
        iota_in = nc.dram_tensor("iota", [P, P], f16, kind="ExternalInput")
        dlw_in = nc.dram_tensor("dloc", [TPC * P, KU], f32, kind="ExternalInput")
    else:
        iota_in = nc.dram_tensor("iota2048", [P, 8 * P], f16, kind="ExternalInput")
        dlw_in = nc.dram_tensor("dlw_in", [TPC * P, KU], f16, kind="ExternalInput")
    idx_in = nc.dram_tensor("idx_in", [TPC, P, KU * 8], i16, kind="ExternalInput")
    oh_in = nc.dram_tensor("oh", [TPC * P, KU * P], f16, kind="ExternalInput")
    out_t = nc.dram_tensor("out", [TPC * P, D], f32, kind="ExternalOutput")

    kv_table = nc.dram_tensor("kv_table", [NT * P, 256], f16)

    NB = 4
    assert NT % NB == 0
    groups = [(g * 8, min(8, KU - g * 8)) for g in range((KU + 7) // 8)]

    with tile.TileContext(nc) as tc:
        with (
            tc.tile_pool(name="const", bufs=1) as cp,
            tc.tile_pool(name="sbuf", bufs=4) as sb,
            tc.tile_pool(name="med", bufs=3) as mp,
            tc.tile_pool(name="big", bufs=3) as bigp,
            tc.tile_pool(name="psA", bufs=2, space="PSUM") as psA,
            tc.tile_pool(name="psB", bufs=2, space="PSUM") as psB,
        ):
            wkv_sb = cp.tile([D, 256], f16, tag="wkv")
            wqs_sb = cp.tile([D, 256], f16, tag="wqs")
            iota_sb = cp.tile([P, P] if SAFE_COMPUTE else [P, 8 * P], f16,
                              tag="iota")
            q_loc = cp.tile([P, TPC * D], f16, tag="qloc")
            s_loc = cp.tile([P, TPC * D], f16, tag="sloc")
            nc.sync.dma_start(out=wkv_sb[:, 0:128], in_=wkT[:])
            nc.sync.dma_start(out=wkv_sb[:, 128:256], in_=wvT[:])
            nc.sync.dma_start(out=wqs_sb[:, 0:128], in_=wqT[:])
            nc.sync.dma_start(out=wqs_sb[:, 128:256], in_=wsT[:])
            nc.sync.dma_start(out=iota_sb[:], in_=iota_in[:])

            # ---------------- node phase: full kv table ----------------
            for it in range(NT // NB):
                t0 = it * NB
                xT = sb.tile([P, NB * P], f16, tag="xT")
                nc.sync.dma_start(
                    out=xT[:], in_=x_permT[:, t0 * P:(t0 + NB) * P])
                pkv = psA.tile([P, NB * 256], f32, tag="pbig")
                for b in range(NB):
                    nc.tensor.matmul(pkv[:, b * 256:(b + 1) * 256],
                                     lhsT=xT[:, b * P:(b + 1) * P], rhs=wkv_sb[:],
                                     start=True, stop=True)
                kvt = sb.tile([P, NB * 256], f16, tag="kvt")
                nc.scalar.activation(out=kvt[:], in_=pkv[:], func=COPY)
                nc.sync.dma_start(
                    out=kv_table[t0 * P:(t0 + NB) * P, :].rearrange(
                        "(b p) c -> p b c", p=P),
                    in_=kvt[:].rearrange("p (b c) -> p b c", c=256))

            # ---------------- local phase: q and skip ----------------
            u = 0
            while u < TPC:
                lb = min(NB, TPC - u)
                xTl = sb.tile([P, NB * P], f16, tag="xT")
                nc.sync.dma_start(
                    out=xTl[:, :lb * P], in_=x_locT[:, u * P:(u + lb) * P])
                pq = psA.tile([P, NB * 256], f32, tag="pbig")
                for b in range(lb):
                    nc.tensor.matmul(pq[:, b * 256:(b + 1) * 256],
                                     lhsT=xTl[:, b * P:(b + 1) * P], rhs=wqs_sb[:],
                                     start=True, stop=True)
                nc.scalar.activation(
                    out=q_loc[:, u * D:(u + lb) * D].rearrange(
                        "p (b c) -> p b c", c=P),
                    in_=pq[:, :lb * 256].rearrange(
                        "p (b c) -> p b c", c=256)[:, :, 0:128], func=COPY)
                xl = sb.tile([P, NB, P], f16, tag="xl")
                nc.sync.dma_start(
                    out=xl[:, :lb, :],
                    in_=x_loc[u * P:(u + lb) * P, :].rearrange(
                        "(b p) c -> p b c", p=P))
                nc.vector.tensor_tensor(
                    out=s_loc[:, u * D:(u + lb) * D].rearrange(
                        "p (b c) -> p b c", c=P),
                    in0=pq[:, :lb * 256].rearrange(
                        "p (b c) -> p b c", c=256)[:, :, 128:256],
                    in1=xl[:, :lb, :], op=ADD)
                u += lb

            # ---------------- edge phase ----------------
            for u in range(TPC):
                idx = mp.tile([P, KU * 8], i16, tag="idx")
                nc.sync.dma_start(out=idx[:], in_=idx_in[u, :, :])
                dlw = mp.tile([P, KU], f32 if SAFE_COMPUTE else f16, tag="dlw")
                nc.sync.dma_start(out=dlw[:], in_=dlw_in[u * P:(u + 1) * P, :])
                kv_g = bigp.tile([P, KU, 256], f16, tag="kvg")
                # dma_gather caps at 1024 idxs/call: split each half into
                # <=8-chunk sub-gathers
                for (h0, hk, tbl) in ((0, KA, kv_table[0:HALF, :]),
                                      (KA, KB, kv_table[HALF:NT * P, :])):
                    for s0 in range(0, hk, 8):
                        sk = min(8, hk - s0)
                        j0 = h0 + s0
                        nc.gpsimd.dma_gather(
                            out_ap=kv_g[:, j0:j0 + sk, :],
                            in_ap=tbl,
                            idxs_ap=idx[:, j0 * 8:(j0 + sk) * 8],
                            num_idxs=sk * P, num_idxs_reg=sk * P,
                            elem_size=256)
                oh = bigp.tile([P, KU * P], f16, tag="oh")
                nc.sync.dma_start(out=oh[:], in_=oh_in[u * P:(u + 1) * P, :])

                psS = psB.tile([P, 132], f32, tag="acc")
                for (j0, gsz) in groups:
                    pqe = psA.tile([P, NB * 256], f32, tag="pbig")
                    for jj in range(gsz):
                        j = j0 + jj
                        nc.tensor.matmul(
                            pqe[:, jj * P:(jj + 1) * P],
                            lhsT=oh[:, j * P:(j + 1) * P],
                            rhs=q_loc[:, u * D:(u + 1) * D],
                            start=True, stop=True)
                    if SAFE_COMPUTE:
                        ohT = sb.tile([P, 8, P], f16, tag="ohT")
                        for jj in range(gsz):
                            j = j0 + jj
                            nc.vector.tensor_scalar(
                                out=ohT[:, jj, :], in0=iota_sb[:],
                                scalar1=dlw[:, j:j + 1], scalar2=None, op0=ISEQ)
                        qk = sb.tile([P, 8, P], f16, tag="qk")
                        nc.vector.tensor_tensor(
                            out=qk[:, :gsz, :],
                            in0=pqe[:, :gsz * P].rearrange("p (a c) -> p a c", c=P),
                            in1=kv_g[:, j0:j0 + gsz, 0:128], op=MUL)
                        t16 = sb.tile([P, 8 * H, 16], f16, tag="t16")
                        qkv = qk[:, :gsz, :].rearrange(
                            "p a (h e) -> p (a h) e", e=DH)
                        nc.vector.tensor_tensor(out=t16[:, :gsz * H, :],
                                                in0=qkv[:, :, 0:16],
                                                in1=qkv[:, :, 16:32], op=ADD)
                        t8 = sb.tile([P, 8 * H, 8], f16, tag="t8")
                        nc.vector.tensor_tensor(out=t8[:, :gsz * H, :],
                                                in0=t16[:, :gsz * H, 0:8],
                                                in1=t16[:, :gsz * H, 8:16], op=ADD)
                        t4 = sb.tile([P, 8 * H, 4], f16, tag="t4")
                        nc.vector.tensor_tensor(out=t4[:, :gsz * H, :],
                                                in0=t8[:, :gsz * H, 0:4],
                                                in1=t8[:, :gsz * H, 4:8], op=ADD)
                        t2 = sb.tile([P, 8 * H, 2], f16, tag="t2")
                        nc.vector.tensor_tensor(out=t2[:, :gsz * H, :],
                                                in0=t4[:, :gsz * H, 0:2],
                                                in1=t4[:, :gsz * H, 2:4], op=ADD)
                        alpha = sb.tile([P, 8 * H, 1], f16, tag="alpha")
                        nc.vector.tensor_tensor(out=alpha[:, :gsz * H, :],
                                                in0=t2[:, :gsz * H, 0:1],
                                                in1=t2[:, :gsz * H, 1:2], op=ADD)
                        X = sb.tile([P, 8, 132], f16, tag="X")
                        nc.scalar.activation(
                            out=X[:, :gsz, 128:132],
                            in_=alpha[:, :gsz * H, 0:1].rearrange(
                                "p (a h) e -> p a (h e)", h=H),
                            func=EXP)
                        nc.vector.tensor_tensor(
                            out=X[:, :gsz, 0:128].rearrange(
                                "p a (h e) -> p a h e", e=DH),
                            in0=kv_g[:, j0:j0 + gsz, 128:256].rearrange(
                                "p a (h e) -> p a h e", e=DH),
                            in1=X[:, :gsz, 128:132, None].to_broadcast(
                                [P, gsz, H, DH]),
                            op=MUL)
                    else:
                        # one-hot (transposed) for the scatter matmul, one ISEQ
                        ohT = sb.tile([P, 8, P], f16, tag="ohT")
                        nc.vector.tensor_tensor(
                            out=ohT[:, :gsz, :],
                            in0=iota_sb[:].rearrange(
                                "p (a c) -> p a c", c=P)[:, :gsz, :],
                            in1=dlw[:, j0:j0 + gsz, None].to_broadcast(
                                [P, gsz, P]),
                            op=ISEQ)
                        # qk product and per-head reduce -> alpha [P, gsz*H] f32
                        qk = sb.tile([P, 8, P], f16, tag="qk")
                        nc.vector.tensor_tensor(
                            out=qk[:, :gsz, :],
                            in0=pqe[:, :gsz * P].rearrange("p (a c) -> p a c", c=P),
                            in1=kv_g[:, j0:j0 + gsz, 0:128], op=MUL)
                        alpha = sb.tile([P, 8 * H], f32, tag="alpha")
                        nc.vector.tensor_reduce(
                            out=alpha[:, :gsz * H],
                            in_=qk[:, :gsz, :].rearrange(
                                "p a (h e) -> p (a h) e", e=DH),
                            axis=AXX, op=ADD)
                        # exp + broadcast across head dims on the scalar engine
                        Xa = sb.tile([P, 8, P], f16, tag="Xa")
                        nc.scalar.activation(
                            out=Xa[:, :gsz, :].rearrange(
                                "p a (h e) -> p a h e", e=DH),
                            in_=alpha[:, :gsz * H].rearrange(
                                "p (a h) -> p a h", h=H)[:, :, :, None].to_broadcast(
                                [P, gsz, H, DH]),
                            func=EXP)
                        X = sb.tile([P, 8, 132], f16, tag="X")
                        nc.scalar.activation(
                            out=X[:, :gsz, 128:132],
                            in_=alpha[:, :gsz * H].rearrange(
                                "p (a h) -> p a h", h=H),
                            func=EXP)
                        nc.vector.tensor_tensor(
                            out=X[:, :gsz, 0:128],
                            in0=kv_g[:, j0:j0 + gsz, 128:256],
                            in1=Xa[:, :gsz, :], op=MUL)
                    for jj in range(gsz):
                        j = j0 + jj
                        nc.tensor.matmul(
                            psS[:, 0:132], lhsT=ohT[:, jj, :], rhs=X[:, jj, 0:132],
                            start=(j == 0), stop=(j == KU - 1))
                dn = sb.tile([P, H], f32, tag="dn")
                nc.vector.tensor_scalar(out=dn[:], in0=psS[:, 128:132],
                                        scalar1=1e-16, scalar2=None, op0=ADD)
                rc = sb.tile([P, H], f32, tag="rc")
                nc.vector.reciprocal(out=rc[:], in_=dn[:])
                ot = sb.tile([P, D], f32, tag="ot")
                nc.vector.tensor_tensor(
                    out=ot[:].rearrange("p (h e) -> p h e", e=DH),
                    in0=psS[:, 0:128].rearrange("p (h e) -> p h e", e=DH),
                    in1=rc[:, :, None].to_broadcast([P, H, DH]), op=MUL)
                of = sb.tile([P, D], f32, tag="of")
                nc.vector.tensor_tensor(
                    out=of[:], in0=ot[:], in1=s_loc[:, u * D:(u + 1) * D], op=ADD)
                nc.sync.dma_start(out=out_t[u * P:(u + 1) * P, :], in_=of[:])

    nc.finalize()
    return nc


def _run(inputs, trace=False):
    _ensure_hooks()
    from concourse.bass_utils import run_bass_kernel_spmd

    meta = _prep(**inputs)
    key = (meta['TPC'], meta['NT'], meta['KA'], meta['KB'], meta['HALF'],
           SAFE_COMPUTE)
    if key not in _prog_cache:
        _prog_cache[key] = _build(*key[:5])
    nc = _prog_cache[key]
    res = run_bass_kernel_spmd(nc, meta['in_maps'],
                               core_ids=list(range(NCORES)), trace=trace)
    outs = [res.results[c]["out"] for c in range(NCORES)]
    out_perm = np.concatenate(outs, axis=0)
    out = out_perm[meta['node_slot']].astype(np.float32)
    return out, res


def kernel(**inputs) -> np.ndarray:
    out, _ = _run(inputs, trace=False)
    return out


# revision 23
# speedup vs baseline: 3.7017x; 2.4616x over previous
"""TransformerConv MixerBlock (x + TransformerConv(x, edge_index)) on 8 trn2 NeuronCores.

Strategy (v4): permute+bin-pack nodes into 128-node tiles balanced by
in-degree (49 tiles/core). The host prepares x in EDGE ORDER, transposed
(x_edgeT: column e = x[src of edge e]) — a pure permutation, so the device
never does a random-access gather (SWDGE Q7 descriptor generation was the
bottleneck in gather-based versions at ~5-10 ns/row). Each core computes, per
128-edge chunk: [k|v] = x_edgeT_chunk^T @ [WkT|WvT] (dense matmul), q per
edge via one-hot matmul against the tile's q, per-head dots + segment softmax
(denominator accumulated via a ones column), and scatter-accumulates
exp(alpha)*[v|1] into PSUM with a one-hot-transposed matmul; then normalize +
skip + residual. PSUM->SBUF kv copies run on the otherwise-idle GpSimd
engine; exp+head-broadcast on the scalar engine so the v-multiply runs in DVE
2x mode.
"""
import sys, os, types, math, heapq
sys.path.insert(0, '/opt/trn_rl_repo')
import numpy as np

P = 128
D = 128
H = 4
DH = 32
NCORES = 8

_prog_cache = {}


def _ensure_hooks():
    """Best-effort shim of antenv.axon_hooks so trace=True profiling works."""
    try:
        import antenv
        if 'antenv.axon_hooks' not in sys.modules:
            mod = types.ModuleType('antenv.axon_hooks')
            state = {'hook': None}
            mod.set_axon_ntff_profile_hook = lambda h: state.__setitem__('hook', h)
            mod.get_axon_ntff_profile_hook = lambda: state['hook']
            sys.modules['antenv.axon_hooks'] = mod
            antenv.axon_hooks = mod
            from trn_agent_boot.trn_boot import _ntff_profile_via_ctypes
            hook = _ntff_profile_via_ctypes('/opt/axon/libaxon_pjrt.so')
            if hook is not None:
                mod.set_axon_ntff_profile_hook(hook)
    except Exception:
        pass
    try:
        import concourse.bass_utils as bass_utils
        bass_utils.upload_artifacts = lambda tmpdir: tmpdir
    except Exception:
        pass


def _prep(x, edge_index, Wq, bq, Wk, bk, Wv, bv, Wskip, bskip):
    N = x.shape[0]
    E = edge_index.shape[1]
    TPC = (N + NCORES * P - 1) // (NCORES * P)
    NT = NCORES * TPC

    src = np.asarray(edge_index[0], dtype=np.int64)
    dst = np.asarray(edge_index[1], dtype=np.int64)
    deg = np.bincount(dst, minlength=N)

    # --- bin-pack nodes into NT tiles of <=P nodes, balancing degree sums ---
    order = np.argsort(-deg, kind='stable')
    heap = [(0, t) for t in range(NT)]
    heapq.heapify(heap)
    counts = np.zeros(NT, dtype=np.int64)
    tile_deg = np.zeros(NT, dtype=np.int64)
    node_slot = np.empty(N, dtype=np.int64)
    for n in order:
        while True:
            dsum, t = heapq.heappop(heap)
            if counts[t] < P:
                break
        node_slot[n] = t * P + counts[t]
        counts[t] += 1
        tile_deg[t] += deg[n]
        if counts[t] < P:
            heapq.heappush(heap, (dsum + int(deg[n]), t))
    KU = max(1, int((tile_deg.max() + P - 1) // P))

    # --- permuted node features ---
    x_perm = np.zeros((NT * P, D), dtype=np.float16)
    x_perm[node_slot] = np.asarray(x, dtype=np.float16)
    x_permT = x_perm.T.copy()

    # --- per-tile edge lists (sorted by src slot for locality) ---
    src_slot = node_slot[src]
    dst_slot = node_slot[dst]
    et = dst_slot // P
    key = et * (1 << 32) + src_slot
    eorder = np.argsort(key, kind='stable')
    et_s = et[eorder]
    src_s = src_slot[eorder]
    dloc_s = dst_slot[eorder] - et_s * P

    ecnt = np.bincount(et, minlength=NT)
    eoff = np.zeros(NT + 1, dtype=np.int64)
    np.cumsum(ecnt, out=eoff[1:])
    pos = np.arange(E) - eoff[et_s]

    # padded per-tile edge arrays: slot (tile, chunk j, part p) = edge j*128+p
    src_pad = np.zeros(NT * KU * P, dtype=np.int64)
    dl_pad = np.full(NT * KU * P, 255, dtype=np.int64)
    flat = et_s * (KU * P) + pos
    src_pad[flat] = src_s
    dl_pad[flat] = dloc_s

    # x in edge order, transposed: [D, NT*KU*P]
    x_edgeT = x_perm[src_pad].T.copy()

    # one-hot [tile, P(row n), KU*P(col)] and biased dloc for the ISEQ build
    dl3 = dl_pad.reshape(NT, KU, P)
    oh_all = (dl_pad.reshape(NT, 1, KU * P)
              == np.arange(P).reshape(1, P, 1)).astype(np.float16)
    dlw_all = (dl3.transpose(0, 2, 1)
               + 256 * (np.arange(KU) % 8).reshape(1, 1, KU)).astype(np.float16)

    s = 1.0 / math.sqrt(DH)
    wkT = np.asarray(Wk, dtype=np.float32).T.astype(np.float16).copy()
    wvT = np.asarray(Wv, dtype=np.float32).T.astype(np.float16).copy()
    wqT = (np.asarray(Wq, dtype=np.float32).T * s).astype(np.float16).copy()
    wsT = np.asarray(Wskip, dtype=np.float32).T.astype(np.float16).copy()
    for b in (bq, bk, bv, bskip):
        assert np.abs(np.asarray(b)).max() == 0.0, "nonzero biases not supported"
    # iota blocks: iota2048[p, a*128+c] = a*256 + c  (for ISEQ ohT build)
    io = (256 * np.arange(8).reshape(8, 1) + np.arange(P).reshape(1, P))
    iota2048 = np.tile(io.reshape(1, 8 * P), (P, 1)).astype(np.float16).copy()

    in_maps = []
    for c in range(NCORES):
        t0, t1 = c * TPC, (c + 1) * TPC
        in_maps.append({
            "x_loc": x_perm[t0 * P:t1 * P].copy(),
            "x_locT": x_permT[:, t0 * P:t1 * P].copy(),
            "x_edgeT": x_edgeT[:, t0 * KU * P:t1 * KU * P].copy(),
            "wkT": wkT, "wvT": wvT, "wqT": wqT, "wsT": wsT,
            "iota2048": iota2048,
            "dlw_in": dlw_all[t0:t1].reshape(TPC * P, KU).copy(),
            "oh": oh_all[t0:t1].reshape(TPC * P, KU * P).copy(),
        })
    return dict(N=N, E=E, TPC=TPC, NT=NT, KU=KU,
                node_slot=node_slot, in_maps=in_maps)


def _build(TPC, NT, KU):
    import concourse.bass as bass
    import concourse.bacc as bacc
    import concourse.mybir as mybir
    import concourse.tile as tile

    f16 = mybir.dt.float16
    f32 = mybir.dt.float32
    MUL = mybir.AluOpType.mult
    ADD = mybir.AluOpType.add
    ISEQ = mybir.AluOpType.is_equal
    EXP = mybir.ActivationFunctionType.Exp
    COPY = mybir.ActivationFunctionType.Copy
    AXX = mybir.AxisListType.X

    nc = bacc.Bacc("TRN2", target_bir_lowering=False, debug=False)
    x_loc = nc.dram_tensor("x_loc", [TPC * P, D], f16, kind="ExternalInput")
    x_locT = nc.dram_tensor("x_locT", [D, TPC * P], f16, kind="ExternalInput")
    x_edgeT = nc.dram_tensor("x_edgeT", [D, TPC * KU * P], f16,
                             kind="ExternalInput")
    wkT = nc.dram_tensor("wkT", [D, D], f16, kind="ExternalInput")
    wvT = nc.dram_tensor("wvT", [D, D], f16, kind="ExternalInput")
    wqT = nc.dram_tensor("wqT", [D, D], f16, kind="ExternalInput")
    wsT = nc.dram_tensor("wsT", [D, D], f16, kind="ExternalInput")
    iota_in = nc.dram_tensor("iota2048", [P, 8 * P], f16, kind="ExternalInput")
    dlw_in = nc.dram_tensor("dlw_in", [TPC * P, KU], f16, kind="ExternalInput")
    oh_in = nc.dram_tensor("oh", [TPC * P, KU * P], f16, kind="ExternalInput")
    out_t = nc.dram_tensor("out", [TPC * P, D], f32, kind="ExternalOutput")

    NB = 4
    groups = [(g * 8, min(8, KU - g * 8)) for g in range((KU + 7) // 8)]
    kvsub = [(s0 * 4, min(4, KU - s0 * 4)) for s0 in range((KU + 3) // 4)]

    with tile.TileContext(nc) as tc:
        with (
            tc.tile_pool(name="const", bufs=1) as cp,
            tc.tile_pool(name="sbuf", bufs=4) as sb,
            tc.tile_pool(name="med", bufs=3) as mp,
            tc.tile_pool(name="big", bufs=3) as bigp,
            tc.tile_pool(name="psA", bufs=2, space="PSUM") as psA,
            tc.tile_pool(name="psB", bufs=2, space="PSUM") as psB,
        ):
            wkv_sb = cp.tile([D, 256], f16, tag="wkv")
            wqs_sb = cp.tile([D, 256], f16, tag="wqs")
            iota_sb = cp.tile([P, 8 * P], f16, tag="iota")
            q_loc = cp.tile([P, TPC * D], f16, tag="qloc")
            s_loc = cp.tile([P, TPC * D], f16, tag="sloc")
            nc.sync.dma_start(out=wkv_sb[:, 0:128], in_=wkT[:])
            nc.sync.dma_start(out=wkv_sb[:, 128:256], in_=wvT[:])
            nc.sync.dma_start(out=wqs_sb[:, 0:128], in_=wqT[:])
            nc.sync.dma_start(out=wqs_sb[:, 128:256], in_=wsT[:])
            nc.sync.dma_start(out=iota_sb[:], in_=iota_in[:])

            # ---------------- local phase: q and skip ----------------
            u = 0
            while u < TPC:
                lb = min(NB, TPC - u)
                xTl = sb.tile([P, NB * P], f16, tag="xT")
                nc.sync.dma_start(
                    out=xTl[:, :lb * P], in_=x_locT[:, u * P:(u + lb) * P])
                pq = psA.tile([P, NB * 256], f32, tag="pbig")
                for b in range(lb):
                    nc.tensor.matmul(pq[:, b * 256:(b + 1) * 256],
                                     lhsT=xTl[:, b * P:(b + 1) * P], rhs=wqs_sb[:],
                                     start=True, stop=True)
                nc.scalar.activation(
                    out=q_loc[:, u * D:(u + lb) * D].rearrange(
                        "p (b c) -> p b c", c=P),
                    in_=pq[:, :lb * 256].rearrange(
                        "p (b c) -> p b c", c=256)[:, :, 0:128], func=COPY)
                xl = sb.tile([P, NB, P], f16, tag="xl")
                nc.sync.dma_start(
                    out=xl[:, :lb, :],
                    in_=x_loc[u * P:(u + lb) * P, :].rearrange(
                        "(b p) c -> p b c", p=P))
                nc.vector.tensor_tensor(
                    out=s_loc[:, u * D:(u + lb) * D].rearrange(
                        "p (b c) -> p b c", c=P),
                    in0=pq[:, :lb * 256].rearrange(
                        "p (b c) -> p b c", c=256)[:, :, 128:256],
                    in1=xl[:, :lb, :], op=ADD)
                u += lb

            # ---------------- edge phase ----------------
            for u in range(TPC):
                xeT = bigp.tile([P, KU * P], f16, tag="xeT")
                nc.sync.dma_start(
                    out=xeT[:], in_=x_edgeT[:, u * KU * P:(u + 1) * KU * P])
                dlw = mp.tile([P, KU], f16, tag="dlw")
                nc.sync.dma_start(out=dlw[:], in_=dlw_in[u * P:(u + 1) * P, :])
                oh = bigp.tile([P, KU * P], f16, tag="oh")
                nc.sync.dma_start(out=oh[:], in_=oh_in[u * P:(u + 1) * P, :])

                # per-edge [k|v] via dense matmul; PSUM -> SBUF f16 on gpsimd
                kv_sb = bigp.tile([P, KU, 256], f16, tag="kvsb")
                for (c0, csz) in kvsub:
                    pkv = psA.tile([P, NB * 256], f32, tag="pbig")
                    for cc in range(csz):
                        j = c0 + cc
                        nc.tensor.matmul(
                            pkv[:, cc * 256:(cc + 1) * 256],
                            lhsT=xeT[:, j * P:(j + 1) * P], rhs=wkv_sb[:],
                            start=True, stop=True)
                    nc.scalar.activation(
                        out=kv_sb[:, c0:c0 + csz, :],
                        in_=pkv[:, :csz * 256].rearrange(
                            "p (b c) -> p b c", c=256),
                        func=COPY)

                psS = psB.tile([P, 132], f32, tag="acc")
                for (j0, gsz) in groups:
                    pqe = psA.tile([P, NB * 256], f32, tag="pbig")
                    for jj in range(gsz):
                        j = j0 + jj
                        nc.tensor.matmul(
                            pqe[:, jj * P:(jj + 1) * P],
                            lhsT=oh[:, j * P:(j + 1) * P],
                            rhs=q_loc[:, u * D:(u + 1) * D],
                            start=True, stop=True)
                    # one-hot (transposed) for the scatter matmul, one ISEQ
                    ohT = sb.tile([P, 8, P], f16, tag="ohT")
                    nc.vector.tensor_tensor(
                        out=ohT[:, :gsz, :],
                        in0=iota_sb[:].rearrange(
                            "p (a c) -> p a c", c=P)[:, :gsz, :],
                        in1=dlw[:, j0:j0 + gsz, None].to_broadcast([P, gsz, P]),
                        op=ISEQ)
                    # qk product and per-head reduce -> alpha [P, gsz*H] f32
                    qk = sb.tile([P, 8, P], f16, tag="qk")
                    nc.vector.tensor_tensor(
                        out=qk[:, :gsz, :],
                        in0=pqe[:, :gsz * P].rearrange("p (a c) -> p a c", c=P),
                        in1=kv_sb[:, j0:j0 + gsz, 0:128], op=MUL)
                    alpha = sb.tile([P, 8 * H], f32, tag="alpha")
                    nc.vector.tensor_reduce(
                        out=alpha[:, :gsz * H],
                        in_=qk[:, :gsz, :].rearrange(
                            "p a (h e) -> p (a h) e", e=DH),
                        axis=AXX, op=ADD)
                    # exp + broadcast across head dims on the scalar engine
                    Xa = sb.tile([P, 8, P], f16, tag="Xa")
                    nc.scalar.activation(
                        out=Xa[:, :gsz, :].rearrange(
                            "p a (h e) -> p a h e", e=DH),
                        in_=alpha[:, :gsz * H].rearrange(
                            "p (a h) -> p a h", h=H)[:, :, :, None].to_broadcast(
                            [P, gsz, H, DH]),
                        func=EXP)
                    X = sb.tile([P, 8, 132], f16, tag="X")
                    nc.scalar.activation(
                        out=X[:, :gsz, 128:132],
                        in_=alpha[:, :gsz * H].rearrange("p (a h) -> p a h", h=H),
                        func=EXP)
                    nc.vector.tensor_tensor(
                        out=X[:, :gsz, 0:128],
                        in0=kv_sb[:, j0:j0 + gsz, 128:256],
                        in1=Xa[:, :gsz, :], op=MUL)
                    for jj in range(gsz):
                        j = j0 + jj
                        nc.tensor.matmul(
                            psS[:, 0:132], lhsT=ohT[:, jj, :], rhs=X[:, jj, 0:132],
                            start=(j == 0), stop=(j == KU - 1))
                dn = sb.tile([P, H], f32, tag="dn")
                nc.vector.tensor_scalar(out=dn[:], in0=psS[:, 128:132],
                                        scalar1=1e-16, scalar2=None, op0=ADD)
                rc = sb.tile([P, H], f32, tag="rc")
                nc.vector.reciprocal(out=rc[:], in_=dn[:])
                ot = sb.tile([P, D], f32, tag="ot")
                nc.vector.tensor_tensor(
                    out=ot[:].rearrange("p (h e) -> p h e", e=DH),
                    in0=psS[:, 0:128].rearrange("p (h e) -> p h e", e=DH),
                    in1=rc[:, :, None].to_broadcast([P, H, DH]), op=MUL)
                of = sb.tile([P, D], f32, tag="of")
                nc.vector.tensor_tensor(
                    out=of[:], in0=ot[:], in1=s_loc[:, u * D:(u + 1) * D], op=ADD)
                nc.sync.dma_start(out=out_t[u * P:(u + 1) * P, :], in_=of[:])

    nc.finalize()
    return nc


def _run(inputs, trace=False):
    _ensure_hooks()
    from concourse.bass_utils import run_bass_kernel_spmd

    meta = _prep(**inputs)
    key = (meta['TPC'], meta['NT'], meta['KU'])
    if key not in _prog_cache:
        _prog_cache[key] = _build(*key)
    nc = _prog_cache[key]
    res = run_bass_kernel_spmd(nc, meta['in_maps'],
                               core_ids=list(range(NCORES)), trace=trace)
    outs = [res.results[c]["out"] for c in range(NCORES)]
    out_perm = np.concatenate(outs, axis=0)
    out = out_perm[meta['node_slot']].astype(np.float32)
    return out, res


def kernel(**inputs) -> np.ndarray:
    out, _ = _run(inputs, trace=False)
    return out


# revision 29
# speedup vs baseline: 3.9293x; 1.0615x over previous
"""TransformerConv MixerBlock (x + TransformerConv(x, edge_index)) on 8 trn2 NeuronCores.

Strategy (v4): permute+bin-pack nodes into 128-node tiles balanced by
in-degree (49 tiles/core). The host prepares x in EDGE ORDER, transposed
(x_edgeT: column e = x[src of edge e]) — a pure permutation, so the device
never does a random-access gather (SWDGE Q7 descriptor generation was the
bottleneck in gather-based versions at ~5-10 ns/row). Each core computes, per
128-edge chunk: [k|v] = x_edgeT_chunk^T @ [WkT|WvT] (dense matmul), q per
edge via one-hot matmul against the tile's q, per-head dots + segment softmax
(denominator accumulated via a ones column), and scatter-accumulates
exp(alpha)*[v|1] into PSUM with a one-hot-transposed matmul; then normalize +
skip + residual. PSUM->SBUF kv copies run on the otherwise-idle GpSimd
engine; exp+head-broadcast on the scalar engine so the v-multiply runs in DVE
2x mode.
"""
import sys, os, types, math, heapq
sys.path.insert(0, '/opt/trn_rl_repo')
import numpy as np

P = 128
D = 128
H = 4
DH = 32
NCORES = 8

_prog_cache = {}


def _ensure_hooks():
    """Best-effort shim of antenv.axon_hooks so trace=True profiling works."""
    try:
        import antenv
        if 'antenv.axon_hooks' not in sys.modules:
            mod = types.ModuleType('antenv.axon_hooks')
            state = {'hook': None}
            mod.set_axon_ntff_profile_hook = lambda h: state.__setitem__('hook', h)
            mod.get_axon_ntff_profile_hook = lambda: state['hook']
            sys.modules['antenv.axon_hooks'] = mod
            antenv.axon_hooks = mod
            from trn_agent_boot.trn_boot import _ntff_profile_via_ctypes
            hook = _ntff_profile_via_ctypes('/opt/axon/libaxon_pjrt.so')
            if hook is not None:
                mod.set_axon_ntff_profile_hook(hook)
    except Exception:
        pass
    try:
        import concourse.bass_utils as bass_utils
        bass_utils.upload_artifacts = lambda tmpdir: tmpdir
    except Exception:
        pass


def _prep(x, edge_index, Wq, bq, Wk, bk, Wv, bv, Wskip, bskip):
    N = x.shape[0]
    E = edge_index.shape[1]
    TPC = (N + NCORES * P - 1) // (NCORES * P)
    NT = NCORES * TPC

    src = np.asarray(edge_index[0], dtype=np.int64)
    dst = np.asarray(edge_index[1], dtype=np.int64)
    deg = np.bincount(dst, minlength=N)

    # --- bin-pack nodes into NT tiles of <=P nodes, balancing degree sums ---
    order = np.argsort(-deg, kind='stable')
    heap = [(0, t) for t in range(NT)]
    heapq.heapify(heap)
    counts = np.zeros(NT, dtype=np.int64)
    tile_deg = np.zeros(NT, dtype=np.int64)
    node_slot = np.empty(N, dtype=np.int64)
    for n in order:
        while True:
            dsum, t = heapq.heappop(heap)
            if counts[t] < P:
                break
        node_slot[n] = t * P + counts[t]
        counts[t] += 1
        tile_deg[t] += deg[n]
        if counts[t] < P:
            heapq.heappush(heap, (dsum + int(deg[n]), t))
    KU = max(1, int((tile_deg.max() + P - 1) // P))

    # --- permuted node features ---
    x_perm = np.zeros((NT * P, D), dtype=np.float16)
    x_perm[node_slot] = np.asarray(x, dtype=np.float16)
    x_permT = x_perm.T.copy()

    # --- per-tile edge lists (sorted by src slot for locality) ---
    src_slot = node_slot[src]
    dst_slot = node_slot[dst]
    et = dst_slot // P
    key = et * (1 << 32) + src_slot
    eorder = np.argsort(key, kind='stable')
    et_s = et[eorder]
    src_s = src_slot[eorder]
    dloc_s = dst_slot[eorder] - et_s * P

    ecnt = np.bincount(et, minlength=NT)
    eoff = np.zeros(NT + 1, dtype=np.int64)
    np.cumsum(ecnt, out=eoff[1:])
    pos = np.arange(E) - eoff[et_s]

    # padded per-tile edge arrays: slot (tile, chunk j, part p) = edge j*128+p
    src_pad = np.zeros(NT * KU * P, dtype=np.int64)
    dl_pad = np.full(NT * KU * P, 255, dtype=np.int64)
    flat = et_s * (KU * P) + pos
    src_pad[flat] = src_s
    dl_pad[flat] = dloc_s

    # x in edge order, transposed: [D, NT*KU*P]
    x_edgeT = x_perm[src_pad].T.copy()

    # one-hot [tile, P(row n), KU*P(col j*128+e)] for the q-recovery matmul,
    # and its transpose [tile, P(row e), KU*P(col j*128+n)] for the scatter
    dl3 = dl_pad.reshape(NT, KU, P)
    oh_all = (dl_pad.reshape(NT, 1, KU * P)
              == np.arange(P).reshape(1, P, 1)).astype(np.float16)
    oht_all = (dl3[:, :, :, None] == np.arange(P).reshape(1, 1, 1, P))
    oht_all = oht_all.transpose(0, 2, 1, 3).reshape(
        NT, P, KU * P).astype(np.float16)

    s = 1.0 / math.sqrt(DH)
    wkT = np.asarray(Wk, dtype=np.float32).T.astype(np.float16).copy()
    wvT = np.asarray(Wv, dtype=np.float32).T.astype(np.float16).copy()
    wqT = (np.asarray(Wq, dtype=np.float32).T * s).astype(np.float16).copy()
    wsT = np.asarray(Wskip, dtype=np.float32).T.astype(np.float16).copy()
    for b in (bq, bk, bv, bskip):
        assert np.abs(np.asarray(b)).max() == 0.0, "nonzero biases not supported"

    in_maps = []
    for c in range(NCORES):
        t0, t1 = c * TPC, (c + 1) * TPC
        in_maps.append({
            "x_loc": x_perm[t0 * P:t1 * P].copy(),
            "x_locT": x_permT[:, t0 * P:t1 * P].copy(),
            "x_edgeT": x_edgeT[:, t0 * KU * P:t1 * KU * P].copy(),
            "wkT": wkT, "wvT": wvT, "wqT": wqT, "wsT": wsT,
            "oh": oh_all[t0:t1].reshape(TPC * P, KU * P).copy(),
            "oht": oht_all[t0:t1].reshape(TPC * P, KU * P).copy(),
        })
    return dict(N=N, E=E, TPC=TPC, NT=NT, KU=KU,
                node_slot=node_slot, in_maps=in_maps)


def _build(TPC, NT, KU):
    import concourse.bass as bass
    import concourse.bacc as bacc
    import concourse.mybir as mybir
    import concourse.tile as tile

    f16 = mybir.dt.float16
    f32 = mybir.dt.float32
    MUL = mybir.AluOpType.mult
    ADD = mybir.AluOpType.add
    ISEQ = mybir.AluOpType.is_equal
    EXP = mybir.ActivationFunctionType.Exp
    COPY = mybir.ActivationFunctionType.Copy
    AXX = mybir.AxisListType.X

    nc = bacc.Bacc("TRN2", target_bir_lowering=False, debug=False)
    x_loc = nc.dram_tensor("x_loc", [TPC * P, D], f16, kind="ExternalInput")
    x_locT = nc.dram_tensor("x_locT", [D, TPC * P], f16, kind="ExternalInput")
    x_edgeT = nc.dram_tensor("x_edgeT", [D, TPC * KU * P], f16,
                             kind="ExternalInput")
    wkT = nc.dram_tensor("wkT", [D, D], f16, kind="ExternalInput")
    wvT = nc.dram_tensor("wvT", [D, D], f16, kind="ExternalInput")
    wqT = nc.dram_tensor("wqT", [D, D], f16, kind="ExternalInput")
    wsT = nc.dram_tensor("wsT", [D, D], f16, kind="ExternalInput")
    oh_in = nc.dram_tensor("oh", [TPC * P, KU * P], f16, kind="ExternalInput")
    oht_in = nc.dram_tensor("oht", [TPC * P, KU * P], f16, kind="ExternalInput")
    out_t = nc.dram_tensor("out", [TPC * P, D], f32, kind="ExternalOutput")

    NB = 4
    groups = [(g * 8, min(8, KU - g * 8)) for g in range((KU + 7) // 8)]
    kvsub = [(s0 * 4, min(4, KU - s0 * 4)) for s0 in range((KU + 3) // 4)]

    with tile.TileContext(nc) as tc:
        with (
            tc.tile_pool(name="const", bufs=1) as cp,
            tc.tile_pool(name="sbuf", bufs=4) as sb,
            tc.tile_pool(name="med", bufs=3) as mp,
            tc.tile_pool(name="big", bufs=3) as bigp,
            tc.tile_pool(name="psA", bufs=2, space="PSUM") as psA,
            tc.tile_pool(name="psB", bufs=2, space="PSUM") as psB,
        ):
            wkv_sb = cp.tile([D, 256], f16, tag="wkv")
            wqs_sb = cp.tile([D, 256], f16, tag="wqs")
            ones_sb = cp.tile([P, DH], f16, tag="ones")
            q_loc = cp.tile([P, TPC * D], f16, tag="qloc")
            s_loc = cp.tile([P, TPC * D], f16, tag="sloc")
            nc.sync.dma_start(out=wkv_sb[:, 0:128], in_=wkT[:])
            nc.sync.dma_start(out=wkv_sb[:, 128:256], in_=wvT[:])
            nc.sync.dma_start(out=wqs_sb[:, 0:128], in_=wqT[:])
            nc.sync.dma_start(out=wqs_sb[:, 128:256], in_=wsT[:])
            nc.vector.memset(ones_sb[:], 1.0)

            # ---------------- local phase: q and skip ----------------
            u = 0
            while u < TPC:
                lb = min(NB, TPC - u)
                xTl = sb.tile([P, NB * P], f16, tag="xT")
                nc.sync.dma_start(
                    out=xTl[:, :lb * P], in_=x_locT[:, u * P:(u + lb) * P])
                pq = psA.tile([P, NB * 256], f32, tag="pbig")
                for b in range(lb):
                    nc.tensor.matmul(pq[:, b * 256:(b + 1) * 256],
                                     lhsT=xTl[:, b * P:(b + 1) * P], rhs=wqs_sb[:],
                                     start=True, stop=True)
                nc.scalar.activation(
                    out=q_loc[:, u * D:(u + lb) * D].rearrange(
                        "p (b c) -> p b c", c=P),
                    in_=pq[:, :lb * 256].rearrange(
                        "p (b c) -> p b c", c=256)[:, :, 0:128], func=COPY)
                xl = sb.tile([P, NB, P], f16, tag="xl")
                nc.sync.dma_start(
                    out=xl[:, :lb, :],
                    in_=x_loc[u * P:(u + lb) * P, :].rearrange(
                        "(b p) c -> p b c", p=P))
                nc.vector.tensor_tensor(
                    out=s_loc[:, u * D:(u + lb) * D].rearrange(
                        "p (b c) -> p b c", c=P),
                    in0=pq[:, :lb * 256].rearrange(
                        "p (b c) -> p b c", c=256)[:, :, 128:256],
                    in1=xl[:, :lb, :], op=ADD)
                u += lb

            # ---------------- edge phase ----------------
            for u in range(TPC):
                xeT = bigp.tile([P, KU * P], f16, tag="xeT")
                nc.sync.dma_start(
                    out=xeT[:], in_=x_edgeT[:, u * KU * P:(u + 1) * KU * P])
                oh = bigp.tile([P, KU * P], f16, tag="oh")
                nc.sync.dma_start(out=oh[:], in_=oh_in[u * P:(u + 1) * P, :])
                ohT = bigp.tile([P, KU * P], f16, tag="oht")
                nc.sync.dma_start(out=ohT[:], in_=oht_in[u * P:(u + 1) * P, :])

                # per-edge [k|v] via dense matmul; PSUM -> SBUF f16 on gpsimd
                kv_sb = bigp.tile([P, KU, 256], f16, tag="kvsb")
                for (c0, csz) in kvsub:
                    pkv = psA.tile([P, NB * 256], f32, tag="pbig")
                    for cc in range(csz):
                        j = c0 + cc
                        nc.tensor.matmul(
                            pkv[:, cc * 256:(cc + 1) * 256],
                            lhsT=xeT[:, j * P:(j + 1) * P], rhs=wkv_sb[:],
                            start=True, stop=True)
                    nc.scalar.activation(
                        out=kv_sb[:, c0:c0 + csz, :],
                        in_=pkv[:, :csz * 256].rearrange(
                            "p (b c) -> p b c", c=256),
                        func=COPY)

                psS = psB.tile([P, 132], f32, tag="acc")
                for (j0, gsz) in groups:
                    pqe = psA.tile([P, NB * 256], f32, tag="pbig")
                    for jj in range(gsz):
                        j = j0 + jj
                        nc.tensor.matmul(
                            pqe[:, jj * P:(jj + 1) * P],
                            lhsT=oh[:, j * P:(j + 1) * P],
                            rhs=q_loc[:, u * D:(u + 1) * D],
                            start=True, stop=True)
                    # qk product and per-head reduce -> alpha [P, gsz*H] f32
                    qk = sb.tile([P, 8, P], f16, tag="qk")
                    nc.vector.tensor_tensor(
                        out=qk[:, :gsz, :],
                        in0=pqe[:, :gsz * P].rearrange("p (a c) -> p a c", c=P),
                        in1=kv_sb[:, j0:j0 + gsz, 0:128], op=MUL)
                    alpha = sb.tile([P, 8 * H], f32, tag="alpha")
                    nc.vector.tensor_reduce(
                        out=alpha[:, :gsz * H],
                        in_=qk[:, :gsz, :].rearrange(
                            "p a (h e) -> p (a h) e", e=DH),
                        axis=AXX, op=ADD)
                    # exp on the scalar engine; head-dim broadcast on gpsimd
                    X = sb.tile([P, 8, 132], f16, tag="X")
                    nc.scalar.activation(
                        out=X[:, :gsz, 128:132],
                        in_=alpha[:, :gsz * H].rearrange("p (a h) -> p a h", h=H),
                        func=EXP)
                    Xa = sb.tile([P, 8, P], f16, tag="Xa")
                    nc.gpsimd.tensor_tensor(
                        out=Xa[:, :gsz, :].rearrange(
                            "p a (h e) -> p a h e", e=DH),
                        in0=X[:, :gsz, 128:132, None].to_broadcast(
                            [P, gsz, H, DH]),
                        in1=ones_sb[:, None, None, :].to_broadcast(
                            [P, gsz, H, DH]),
                        op=MUL)
                    nc.vector.tensor_tensor(
                        out=X[:, :gsz, 0:128],
                        in0=kv_sb[:, j0:j0 + gsz, 128:256],
                        in1=Xa[:, :gsz, :], op=MUL)
                    for jj in range(gsz):
                        j = j0 + jj
                        nc.tensor.matmul(
                            psS[:, 0:132],
                            lhsT=ohT[:, (j0 + jj) * P:(j0 + jj + 1) * P],
                            rhs=X[:, jj, 0:132],
                            start=(j == 0), stop=(j == KU - 1))
                dn = sb.tile([P, H], f32, tag="dn")
                nc.vector.tensor_scalar(out=dn[:], in0=psS[:, 128:132],
                                        scalar1=1e-16, scalar2=None, op0=ADD)
                rc = sb.tile([P, H], f32, tag="rc")
                nc.vector.reciprocal(out=rc[:], in_=dn[:])
                ot = sb.tile([P, D], f32, tag="ot")
                nc.vector.tensor_tensor(
                    out=ot[:].rearrange("p (h e) -> p h e", e=DH),
                    in0=psS[:, 0:128].rearrange("p (h e) -> p h e", e=DH),
                    in1=rc[:, :, None].to_broadcast([P, H, DH]), op=MUL)
                of = sb.tile([P, D], f32, tag="of")
                nc.vector.tensor_tensor(
                    out=of[:], in0=ot[:], in1=s_loc[:, u * D:(u + 1) * D], op=ADD)
                nc.sync.dma_start(out=out_t[u * P:(u + 1) * P, :], in_=of[:])

    nc.finalize()
    return nc


def _run(inputs, trace=False):
    _ensure_hooks()
    from concourse.bass_utils import run_bass_kernel_spmd

    meta = _prep(**inputs)
    key = (meta['TPC'], meta['NT'], meta['KU'])
    if key not in _prog_cache:
        _prog_cache[key] = _build(*key)
    nc = _prog_cache[key]
    res = run_bass_kernel_spmd(nc, meta['in_maps'],
                               core_ids=list(range(NCORES)), trace=trace)
    outs = [res.results[c]["out"] for c in range(NCORES)]
    out_perm = np.concatenate(outs, axis=0)
    out = out_perm[meta['node_slot']].astype(np.float32)
    return out, res


def kernel(**inputs) -> np.ndarray:
    out, _ = _run(inputs, trace=False)
    return out


# revision 31
# speedup vs baseline: 4.1300x; 1.0511x over previous
"""TransformerConv MixerBlock (x + TransformerConv(x, edge_index)) on 8 trn2 NeuronCores.

Strategy (v4): permute+bin-pack nodes into 128-node tiles balanced by
in-degree (49 tiles/core). The host prepares x in EDGE ORDER, transposed
(x_edgeT: column e = x[src of edge e]) — a pure permutation, so the device
never does a random-access gather (SWDGE Q7 descriptor generation was the
bottleneck in gather-based versions at ~5-10 ns/row). Each core computes, per
128-edge chunk: [k|v] = x_edgeT_chunk^T @ [WkT|WvT] (dense matmul), q per
edge via one-hot matmul against the tile's q, per-head dots + segment softmax
(denominator accumulated via a ones column), and scatter-accumulates
exp(alpha)*[v|1] into PSUM with a one-hot-transposed matmul; then normalize +
skip + residual. PSUM->SBUF kv copies run on the otherwise-idle GpSimd
engine; exp+head-broadcast on the scalar engine so the v-multiply runs in DVE
2x mode.
"""
import sys, os, types, math, heapq
sys.path.insert(0, '/opt/trn_rl_repo')
import numpy as np

P = 128
D = 128
H = 4
DH = 32
NCORES = 8

_prog_cache = {}


def _ensure_hooks():
    """Best-effort shim of antenv.axon_hooks so trace=True profiling works."""
    try:
        import antenv
        if 'antenv.axon_hooks' not in sys.modules:
            mod = types.ModuleType('antenv.axon_hooks')
            state = {'hook': None}
            mod.set_axon_ntff_profile_hook = lambda h: state.__setitem__('hook', h)
            mod.get_axon_ntff_profile_hook = lambda: state['hook']
            sys.modules['antenv.axon_hooks'] = mod
            antenv.axon_hooks = mod
            from trn_agent_boot.trn_boot import _ntff_profile_via_ctypes
            hook = _ntff_profile_via_ctypes('/opt/axon/libaxon_pjrt.so')
            if hook is not None:
                mod.set_axon_ntff_profile_hook(hook)
    except Exception:
        pass
    try:
        import concourse.bass_utils as bass_utils
        bass_utils.upload_artifacts = lambda tmpdir: tmpdir
    except Exception:
        pass


def _prep(x, edge_index, Wq, bq, Wk, bk, Wv, bv, Wskip, bskip):
    N = x.shape[0]
    E = edge_index.shape[1]
    TPC = (N + NCORES * P - 1) // (NCORES * P)
    NT = NCORES * TPC

    src = np.asarray(edge_index[0], dtype=np.int64)
    dst = np.asarray(edge_index[1], dtype=np.int64)
    deg = np.bincount(dst, minlength=N)

    # --- bin-pack nodes into NT tiles of <=P nodes, balancing degree sums ---
    order = np.argsort(-deg, kind='stable')
    heap = [(0, t) for t in range(NT)]
    heapq.heapify(heap)
    counts = np.zeros(NT, dtype=np.int64)
    tile_deg = np.zeros(NT, dtype=np.int64)
    node_slot = np.empty(N, dtype=np.int64)
    for n in order:
        while True:
            dsum, t = heapq.heappop(heap)
            if counts[t] < P:
                break
        node_slot[n] = t * P + counts[t]
        counts[t] += 1
        tile_deg[t] += deg[n]
        if counts[t] < P:
            heapq.heappush(heap, (dsum + int(deg[n]), t))
    KU = max(1, int((tile_deg.max() + P - 1) // P))

    # --- permuted node features ---
    x_perm = np.zeros((NT * P, D), dtype=np.float16)
    x_perm[node_slot] = np.asarray(x, dtype=np.float16)
    x_permT = x_perm.T.copy()

    # --- per-tile edge lists (sorted by src slot for locality) ---
    src_slot = node_slot[src]
    dst_slot = node_slot[dst]
    et = dst_slot // P
    key = et * (1 << 32) + src_slot
    eorder = np.argsort(key, kind='stable')
    et_s = et[eorder]
    src_s = src_slot[eorder]
    dloc_s = dst_slot[eorder] - et_s * P

    ecnt = np.bincount(et, minlength=NT)
    eoff = np.zeros(NT + 1, dtype=np.int64)
    np.cumsum(ecnt, out=eoff[1:])
    pos = np.arange(E) - eoff[et_s]

    # padded per-tile edge arrays: slot (tile, chunk j, part p) = edge j*128+p
    src_pad = np.zeros(NT * KU * P, dtype=np.int64)
    dl_pad = np.full(NT * KU * P, 255, dtype=np.int64)
    flat = et_s * (KU * P) + pos
    src_pad[flat] = src_s
    dl_pad[flat] = dloc_s

    # x in edge order, transposed: [D, NT*KU*P]
    x_edgeT = x_perm[src_pad].T.copy()

    # one-hot [tile, P(row n), KU*P(col j*128+e)] for the q-recovery matmul,
    # and its transpose [tile, P(row e), KU*P(col j*128+n)] for the scatter
    dl3 = dl_pad.reshape(NT, KU, P)
    oh_all = (dl_pad.reshape(NT, 1, KU * P)
              == np.arange(P).reshape(1, P, 1)).astype(np.float16)
    oht_all = (dl3[:, :, :, None] == np.arange(P).reshape(1, 1, 1, P))
    oht_all = oht_all.transpose(0, 2, 1, 3).reshape(
        NT, P, KU * P).astype(np.float16)

    s = 1.0 / math.sqrt(DH)
    wkT = np.asarray(Wk, dtype=np.float32).T.astype(np.float16).copy()
    wvT = np.asarray(Wv, dtype=np.float32).T.astype(np.float16).copy()
    wqT = (np.asarray(Wq, dtype=np.float32).T * s).astype(np.float16).copy()
    wsT = np.asarray(Wskip, dtype=np.float32).T.astype(np.float16).copy()
    for b in (bq, bk, bv, bskip):
        assert np.abs(np.asarray(b)).max() == 0.0, "nonzero biases not supported"

    in_maps = []
    for c in range(NCORES):
        t0, t1 = c * TPC, (c + 1) * TPC
        in_maps.append({
            "x_loc": x_perm[t0 * P:t1 * P].copy(),
            "x_locT": x_permT[:, t0 * P:t1 * P].copy(),
            "x_edgeT": x_edgeT[:, t0 * KU * P:t1 * KU * P].copy(),
            "wkT": wkT, "wvT": wvT, "wqT": wqT, "wsT": wsT,
            "oh": oh_all[t0:t1].reshape(TPC * P, KU * P).copy(),
            "oht": oht_all[t0:t1].reshape(TPC * P, KU * P).copy(),
        })
    return dict(N=N, E=E, TPC=TPC, NT=NT, KU=KU,
                node_slot=node_slot, in_maps=in_maps)


def _build(TPC, NT, KU):
    import concourse.bass as bass
    import concourse.bacc as bacc
    import concourse.mybir as mybir
    import concourse.tile as tile

    f16 = mybir.dt.float16
    f32 = mybir.dt.float32
    MUL = mybir.AluOpType.mult
    ADD = mybir.AluOpType.add
    ISEQ = mybir.AluOpType.is_equal
    EXP = mybir.ActivationFunctionType.Exp
    COPY = mybir.ActivationFunctionType.Copy
    AXX = mybir.AxisListType.X

    nc = bacc.Bacc("TRN2", target_bir_lowering=False, debug=False)
    x_loc = nc.dram_tensor("x_loc", [TPC * P, D], f16, kind="ExternalInput")
    x_locT = nc.dram_tensor("x_locT", [D, TPC * P], f16, kind="ExternalInput")
    x_edgeT = nc.dram_tensor("x_edgeT", [D, TPC * KU * P], f16,
                             kind="ExternalInput")
    wkT = nc.dram_tensor("wkT", [D, D], f16, kind="ExternalInput")
    wvT = nc.dram_tensor("wvT", [D, D], f16, kind="ExternalInput")
    wqT = nc.dram_tensor("wqT", [D, D], f16, kind="ExternalInput")
    wsT = nc.dram_tensor("wsT", [D, D], f16, kind="ExternalInput")
    oh_in = nc.dram_tensor("oh", [TPC * P, KU * P], f16, kind="ExternalInput")
    oht_in = nc.dram_tensor("oht", [TPC * P, KU * P], f16, kind="ExternalInput")
    out_t = nc.dram_tensor("out", [TPC * P, D], f32, kind="ExternalOutput")

    NB = 4
    groups = [(g * 8, min(8, KU - g * 8)) for g in range((KU + 7) // 8)]
    kvsub = [(s0 * 4, min(4, KU - s0 * 4)) for s0 in range((KU + 3) // 4)]

    with tile.TileContext(nc) as tc:
        with (
            tc.tile_pool(name="const", bufs=1) as cp,
            tc.tile_pool(name="sbuf", bufs=4) as sb,
            tc.tile_pool(name="med", bufs=3) as mp,
            tc.tile_pool(name="big", bufs=3) as bigp,
            tc.tile_pool(name="psA", bufs=2, space="PSUM") as psA,
            tc.tile_pool(name="psB", bufs=2, space="PSUM") as psB,
        ):
            wkv_sb = cp.tile([D, 256], f16, tag="wkv")
            wqs_sb = cp.tile([D, 256], f16, tag="wqs")
            ones_sb = cp.tile([P, DH], f16, tag="ones")
            q_loc = cp.tile([P, TPC * D], f16, tag="qloc")
            s_loc = cp.tile([P, TPC * D], f16, tag="sloc")
            nc.sync.dma_start(out=wkv_sb[:, 0:128], in_=wkT[:])
            nc.sync.dma_start(out=wkv_sb[:, 128:256], in_=wvT[:])
            nc.sync.dma_start(out=wqs_sb[:, 0:128], in_=wqT[:])
            nc.sync.dma_start(out=wqs_sb[:, 128:256], in_=wsT[:])
            nc.vector.memset(ones_sb[:], 1.0)

            # ---------------- local phase: q and skip ----------------
            u = 0
            while u < TPC:
                lb = min(NB, TPC - u)
                xTl = sb.tile([P, NB * P], f16, tag="xT")
                nc.sync.dma_start(
                    out=xTl[:, :lb * P], in_=x_locT[:, u * P:(u + lb) * P])
                pq = psA.tile([P, NB * 256], f32, tag="pbig")
                for b in range(lb):
                    nc.tensor.matmul(pq[:, b * 256:(b + 1) * 256],
                                     lhsT=xTl[:, b * P:(b + 1) * P], rhs=wqs_sb[:],
                                     start=True, stop=True)
                nc.scalar.activation(
                    out=q_loc[:, u * D:(u + lb) * D].rearrange(
                        "p (b c) -> p b c", c=P),
                    in_=pq[:, :lb * 256].rearrange(
                        "p (b c) -> p b c", c=256)[:, :, 0:128], func=COPY)
                xl = sb.tile([P, NB, P], f16, tag="xl")
                nc.sync.dma_start(
                    out=xl[:, :lb, :],
                    in_=x_loc[u * P:(u + lb) * P, :].rearrange(
                        "(b p) c -> p b c", p=P))
                nc.vector.tensor_tensor(
                    out=s_loc[:, u * D:(u + lb) * D].rearrange(
                        "p (b c) -> p b c", c=P),
                    in0=pq[:, :lb * 256].rearrange(
                        "p (b c) -> p b c", c=256)[:, :, 128:256],
                    in1=xl[:, :lb, :], op=ADD)
                u += lb

            # ---------------- edge phase ----------------
            for u in range(TPC):
                xeT = bigp.tile([P, KU * P], f16, tag="xeT")
                nc.sync.dma_start(
                    out=xeT[:], in_=x_edgeT[:, u * KU * P:(u + 1) * KU * P])
                oh = bigp.tile([P, KU * P], f16, tag="oh")
                nc.sync.dma_start(out=oh[:], in_=oh_in[u * P:(u + 1) * P, :])
                ohT = bigp.tile([P, KU * P], f16, tag="oht")
                nc.sync.dma_start(out=ohT[:], in_=oht_in[u * P:(u + 1) * P, :])

                # per-edge [k|v] via dense matmul; PSUM -> SBUF f16 on gpsimd
                kv_sb = bigp.tile([P, KU, 256], f16, tag="kvsb")
                for (c0, csz) in kvsub:
                    pkv = psA.tile([P, NB * 256], f32, tag="pbig")
                    for cc in range(csz):
                        j = c0 + cc
                        nc.tensor.matmul(
                            pkv[:, cc * 256:(cc + 1) * 256],
                            lhsT=xeT[:, j * P:(j + 1) * P], rhs=wkv_sb[:],
                            start=True, stop=True)
                    nc.scalar.activation(
                        out=kv_sb[:, c0:c0 + csz, :],
                        in_=pkv[:, :csz * 256].rearrange(
                            "p (b c) -> p b c", c=256),
                        func=COPY)

                psS = psB.tile([P, 132], f32, tag="acc")
                for (j0, gsz) in groups:
                    pqe = psA.tile([P, NB * 256], f32, tag="pbig")
                    for jj in range(gsz):
                        j = j0 + jj
                        nc.tensor.matmul(
                            pqe[:, jj * P:(jj + 1) * P],
                            lhsT=oh[:, j * P:(j + 1) * P],
                            rhs=q_loc[:, u * D:(u + 1) * D],
                            start=True, stop=True)
                    # qk product and per-head reduce -> alpha [P, gsz*H] f32
                    qk = sb.tile([P, 8, P], f16, tag="qk")
                    nc.vector.tensor_tensor(
                        out=qk[:, :gsz, :],
                        in0=pqe[:, :gsz * P].rearrange("p (a c) -> p a c", c=P),
                        in1=kv_sb[:, j0:j0 + gsz, 0:128], op=MUL)
                    alpha = sb.tile([P, 8 * H], f16, tag="alpha")
                    with nc.allow_low_precision("32-term f16 dot, matches v1 tree"):
                        nc.vector.tensor_reduce(
                            out=alpha[:, :gsz * H],
                            in_=qk[:, :gsz, :].rearrange(
                                "p a (h e) -> p (a h) e", e=DH),
                            axis=AXX, op=ADD)
                    # exp on the scalar engine; head-dim broadcast on gpsimd
                    X = sb.tile([P, 8, 132], f16, tag="X")
                    nc.scalar.activation(
                        out=X[:, :gsz, 128:132],
                        in_=alpha[:, :gsz * H].rearrange("p (a h) -> p a h", h=H),
                        func=EXP)
                    Xa = sb.tile([P, 8, P], f16, tag="Xa")
                    nc.gpsimd.tensor_tensor(
                        out=Xa[:, :gsz, :].rearrange(
                            "p a (h e) -> p a h e", e=DH),
                        in0=X[:, :gsz, 128:132, None].to_broadcast(
                            [P, gsz, H, DH]),
                        in1=ones_sb[:, None, None, :].to_broadcast(
                            [P, gsz, H, DH]),
                        op=MUL)
                    nc.vector.tensor_tensor(
                        out=X[:, :gsz, 0:128],
                        in0=kv_sb[:, j0:j0 + gsz, 128:256],
                        in1=Xa[:, :gsz, :], op=MUL)
                    for jj in range(gsz):
                        j = j0 + jj
                        nc.tensor.matmul(
                            psS[:, 0:132],
                            lhsT=ohT[:, (j0 + jj) * P:(j0 + jj + 1) * P],
                            rhs=X[:, jj, 0:132],
                            start=(j == 0), stop=(j == KU - 1))
                dn = sb.tile([P, H], f32, tag="dn")
                nc.vector.tensor_scalar(out=dn[:], in0=psS[:, 128:132],
                                        scalar1=1e-16, scalar2=None, op0=ADD)
                rc = sb.tile([P, H], f32, tag="rc")
                nc.vector.reciprocal(out=rc[:], in_=dn[:])
                ot = sb.tile([P, D], f32, tag="ot")
                for h in range(H):
                    nc.scalar.activation(
                        out=ot[:, h * DH:(h + 1) * DH],
                        in_=psS[:, h * DH:(h + 1) * DH],
                        func=COPY, scale=rc[:, h:h + 1])
                of = sb.tile([P, D], f32, tag="of")
                nc.vector.tensor_tensor(
                    out=of[:], in0=ot[:], in1=s_loc[:, u * D:(u + 1) * D], op=ADD)
                nc.sync.dma_start(out=out_t[u * P:(u + 1) * P, :], in_=of[:])

    nc.finalize()
    return nc


def _run(inputs, trace=False):
    _ensure_hooks()
    from concourse.bass_utils import run_bass_kernel_spmd

    meta = _prep(**inputs)
    key = (meta['TPC'], meta['NT'], meta['KU'])
    if key not in _prog_cache:
        _prog_cache[key] = _build(*key)
    nc = _prog_cache[key]
    res = run_bass_kernel_spmd(nc, meta['in_maps'],
                               core_ids=list(range(NCORES)), trace=trace)
    outs = [res.results[c]["out"] for c in range(NCORES)]
    out_perm = np.concatenate(outs, axis=0)
    out = out_perm[meta['node_slot']].astype(np.float32)
    return out, res


def kernel(**inputs) -> np.ndarray:
    out, _ = _run(inputs, trace=False)
    return out


# revision 32
# speedup vs baseline: 4.1381x; 1.0020x over previous
"""TransformerConv MixerBlock (x + TransformerConv(x, edge_index)) on 8 trn2 NeuronCores.

Strategy (v4): permute+bin-pack nodes into 128-node tiles balanced by
in-degree (49 tiles/core). The host prepares x in EDGE ORDER, transposed
(x_edgeT: column e = x[src of edge e]) — a pure permutation, so the device
never does a random-access gather (SWDGE Q7 descriptor generation was the
bottleneck in gather-based versions at ~5-10 ns/row). Each core computes, per
128-edge chunk: [k|v] = x_edgeT_chunk^T @ [WkT|WvT] (dense matmul), q per
edge via one-hot matmul against the tile's q, per-head dots + segment softmax
(denominator accumulated via a ones column), and scatter-accumulates
exp(alpha)*[v|1] into PSUM with a one-hot-transposed matmul; then normalize +
skip + residual. PSUM->SBUF kv copies run on the otherwise-idle GpSimd
engine; exp+head-broadcast on the scalar engine so the v-multiply runs in DVE
2x mode.
"""
import sys, os, types, math, heapq
sys.path.insert(0, '/opt/trn_rl_repo')
import numpy as np

P = 128
D = 128
H = 4
DH = 32
NCORES = 8

_prog_cache = {}


def _ensure_hooks():
    """Best-effort shim of antenv.axon_hooks so trace=True profiling works."""
    try:
        import antenv
        if 'antenv.axon_hooks' not in sys.modules:
            mod = types.ModuleType('antenv.axon_hooks')
            state = {'hook': None}
            mod.set_axon_ntff_profile_hook = lambda h: state.__setitem__('hook', h)
            mod.get_axon_ntff_profile_hook = lambda: state['hook']
            sys.modules['antenv.axon_hooks'] = mod
            antenv.axon_hooks = mod
            from trn_agent_boot.trn_boot import _ntff_profile_via_ctypes
            hook = _ntff_profile_via_ctypes('/opt/axon/libaxon_pjrt.so')
            if hook is not None:
                mod.set_axon_ntff_profile_hook(hook)
    except Exception:
        pass
    try:
        import concourse.bass_utils as bass_utils
        bass_utils.upload_artifacts = lambda tmpdir: tmpdir
    except Exception:
        pass


def _prep(x, edge_index, Wq, bq, Wk, bk, Wv, bv, Wskip, bskip):
    N = x.shape[0]
    E = edge_index.shape[1]
    TPC = (N + NCORES * P - 1) // (NCORES * P)
    NT = NCORES * TPC

    src = np.asarray(edge_index[0], dtype=np.int64)
    dst = np.asarray(edge_index[1], dtype=np.int64)
    deg = np.bincount(dst, minlength=N)

    # --- bin-pack nodes into NT tiles of <=P nodes, balancing degree sums ---
    order = np.argsort(-deg, kind='stable')
    heap = [(0, t) for t in range(NT)]
    heapq.heapify(heap)
    counts = np.zeros(NT, dtype=np.int64)
    tile_deg = np.zeros(NT, dtype=np.int64)
    node_slot = np.empty(N, dtype=np.int64)
    for n in order:
        while True:
            dsum, t = heapq.heappop(heap)
            if counts[t] < P:
                break
        node_slot[n] = t * P + counts[t]
        counts[t] += 1
        tile_deg[t] += deg[n]
        if counts[t] < P:
            heapq.heappush(heap, (dsum + int(deg[n]), t))
    KU = max(1, int((tile_deg.max() + P - 1) // P))

    # --- permuted node features ---
    x_perm = np.zeros((NT * P, D), dtype=np.float16)
    x_perm[node_slot] = np.asarray(x, dtype=np.float16)
    x_permT = x_perm.T.copy()

    # --- per-tile edge lists (sorted by src slot for locality) ---
    src_slot = node_slot[src]
    dst_slot = node_slot[dst]
    et = dst_slot // P
    key = et * (1 << 32) + src_slot
    eorder = np.argsort(key, kind='stable')
    et_s = et[eorder]
    src_s = src_slot[eorder]
    dloc_s = dst_slot[eorder] - et_s * P

    ecnt = np.bincount(et, minlength=NT)
    eoff = np.zeros(NT + 1, dtype=np.int64)
    np.cumsum(ecnt, out=eoff[1:])
    pos = np.arange(E) - eoff[et_s]

    # padded per-tile edge arrays: slot (tile, chunk j, part p) = edge j*128+p
    src_pad = np.zeros(NT * KU * P, dtype=np.int64)
    dl_pad = np.full(NT * KU * P, 255, dtype=np.int64)
    flat = et_s * (KU * P) + pos
    src_pad[flat] = src_s
    dl_pad[flat] = dloc_s

    # x in edge order, transposed: [D, NT*KU*P]
    x_edgeT = x_perm[src_pad].T.copy()

    # one-hot [tile, P(row n), KU*P(col j*128+e)] for the q-recovery matmul,
    # and its transpose [tile, P(row e), KU*P(col j*128+n)] for the scatter
    dl3 = dl_pad.reshape(NT, KU, P)
    oh_all = (dl_pad.reshape(NT, 1, KU * P)
              == np.arange(P).reshape(1, P, 1)).astype(np.float16)
    oht_all = (dl3[:, :, :, None] == np.arange(P).reshape(1, 1, 1, P))
    oht_all = oht_all.transpose(0, 2, 1, 3).reshape(
        NT, P, KU * P).astype(np.float16)

    s = 1.0 / math.sqrt(DH)
    wkT = np.asarray(Wk, dtype=np.float32).T.astype(np.float16).copy()
    wvT = np.asarray(Wv, dtype=np.float32).T.astype(np.float16).copy()
    wqT = (np.asarray(Wq, dtype=np.float32).T * s).astype(np.float16).copy()
    wsT = np.asarray(Wskip, dtype=np.float32).T.astype(np.float16).copy()
    for b in (bq, bk, bv, bskip):
        assert np.abs(np.asarray(b)).max() == 0.0, "nonzero biases not supported"

    in_maps = []
    for c in range(NCORES):
        t0, t1 = c * TPC, (c + 1) * TPC
        in_maps.append({
            "x_loc": x_perm[t0 * P:t1 * P].copy(),
            "x_locT": x_permT[:, t0 * P:t1 * P].copy(),
            "x_edgeT": x_edgeT[:, t0 * KU * P:t1 * KU * P].copy(),
            "wkT": wkT, "wvT": wvT, "wqT": wqT, "wsT": wsT,
            "oh": oh_all[t0:t1].reshape(TPC * P, KU * P).copy(),
            "oht": oht_all[t0:t1].reshape(TPC * P, KU * P).copy(),
        })
    return dict(N=N, E=E, TPC=TPC, NT=NT, KU=KU,
                node_slot=node_slot, in_maps=in_maps)


def _build(TPC, NT, KU):
    import concourse.bass as bass
    import concourse.bacc as bacc
    import concourse.mybir as mybir
    import concourse.tile as tile

    f16 = mybir.dt.float16
    f32 = mybir.dt.float32
    MUL = mybir.AluOpType.mult
    ADD = mybir.AluOpType.add
    ISEQ = mybir.AluOpType.is_equal
    EXP = mybir.ActivationFunctionType.Exp
    COPY = mybir.ActivationFunctionType.Copy
    AXX = mybir.AxisListType.X

    nc = bacc.Bacc("TRN2", target_bir_lowering=False, debug=False)
    x_loc = nc.dram_tensor("x_loc", [TPC * P, D], f16, kind="ExternalInput")
    x_locT = nc.dram_tensor("x_locT", [D, TPC * P], f16, kind="ExternalInput")
    x_edgeT = nc.dram_tensor("x_edgeT", [D, TPC * KU * P], f16,
                             kind="ExternalInput")
    wkT = nc.dram_tensor("wkT", [D, D], f16, kind="ExternalInput")
    wvT = nc.dram_tensor("wvT", [D, D], f16, kind="ExternalInput")
    wqT = nc.dram_tensor("wqT", [D, D], f16, kind="ExternalInput")
    wsT = nc.dram_tensor("wsT", [D, D], f16, kind="ExternalInput")
    oh_in = nc.dram_tensor("oh", [TPC * P, KU * P], f16, kind="ExternalInput")
    oht_in = nc.dram_tensor("oht", [TPC * P, KU * P], f16, kind="ExternalInput")
    out_t = nc.dram_tensor("out", [TPC * P, D], f32, kind="ExternalOutput")

    NB = 4
    groups = [(g * 8, min(8, KU - g * 8)) for g in range((KU + 7) // 8)]
    kvsub = [(s0 * 4, min(4, KU - s0 * 4)) for s0 in range((KU + 3) // 4)]

    with tile.TileContext(nc) as tc:
        with (
            tc.tile_pool(name="const", bufs=1) as cp,
            tc.tile_pool(name="sbuf", bufs=6) as sb,
            tc.tile_pool(name="med", bufs=4) as mp,
            tc.tile_pool(name="big", bufs=4) as bigp,
            tc.tile_pool(name="psA", bufs=2, space="PSUM") as psA,
            tc.tile_pool(name="psB", bufs=2, space="PSUM") as psB,
        ):
            wkv_sb = cp.tile([D, 256], f16, tag="wkv")
            wqs_sb = cp.tile([D, 256], f16, tag="wqs")
            ones_sb = cp.tile([P, DH], f16, tag="ones")
            q_loc = cp.tile([P, TPC * D], f16, tag="qloc")
            s_loc = cp.tile([P, TPC * D], f16, tag="sloc")
            nc.sync.dma_start(out=wkv_sb[:, 0:128], in_=wkT[:])
            nc.sync.dma_start(out=wkv_sb[:, 128:256], in_=wvT[:])
            nc.sync.dma_start(out=wqs_sb[:, 0:128], in_=wqT[:])
            nc.sync.dma_start(out=wqs_sb[:, 128:256], in_=wsT[:])
            nc.vector.memset(ones_sb[:], 1.0)

            # ---------------- local phase: q and skip ----------------
            u = 0
            while u < TPC:
                lb = min(NB, TPC - u)
                xTl = sb.tile([P, NB * P], f16, tag="xT")
                nc.sync.dma_start(
                    out=xTl[:, :lb * P], in_=x_locT[:, u * P:(u + lb) * P])
                pq = psA.tile([P, NB * 256], f32, tag="pbig")
                for b in range(lb):
                    nc.tensor.matmul(pq[:, b * 256:(b + 1) * 256],
                                     lhsT=xTl[:, b * P:(b + 1) * P], rhs=wqs_sb[:],
                                     start=True, stop=True)
                nc.scalar.activation(
                    out=q_loc[:, u * D:(u + lb) * D].rearrange(
                        "p (b c) -> p b c", c=P),
                    in_=pq[:, :lb * 256].rearrange(
                        "p (b c) -> p b c", c=256)[:, :, 0:128], func=COPY)
                xl = sb.tile([P, NB, P], f16, tag="xl")
                nc.sync.dma_start(
                    out=xl[:, :lb, :],
                    in_=x_loc[u * P:(u + lb) * P, :].rearrange(
                        "(b p) c -> p b c", p=P))
                nc.vector.tensor_tensor(
                    out=s_loc[:, u * D:(u + lb) * D].rearrange(
                        "p (b c) -> p b c", c=P),
                    in0=pq[:, :lb * 256].rearrange(
                        "p (b c) -> p b c", c=256)[:, :, 128:256],
                    in1=xl[:, :lb, :], op=ADD)
                u += lb

            # ---------------- edge phase ----------------
            for u in range(TPC):
                xeT = bigp.tile([P, KU * P], f16, tag="xeT")
                nc.sync.dma_start(
                    out=xeT[:], in_=x_edgeT[:, u * KU * P:(u + 1) * KU * P])
                oh = bigp.tile([P, KU * P], f16, tag="oh")
                nc.sync.dma_start(out=oh[:], in_=oh_in[u * P:(u + 1) * P, :])
                ohT = bigp.tile([P, KU * P], f16, tag="oht")
                nc.sync.dma_start(out=ohT[:], in_=oht_in[u * P:(u + 1) * P, :])

                # per-edge [k|v] via dense matmul; PSUM -> SBUF f16 on gpsimd
                kv_sb = bigp.tile([P, KU, 256], f16, tag="kvsb")
                for (c0, csz) in kvsub:
                    pkv = psA.tile([P, NB * 256], f32, tag="pbig")
                    for cc in range(csz):
                        j = c0 + cc
                        nc.tensor.matmul(
                            pkv[:, cc * 256:(cc + 1) * 256],
                            lhsT=xeT[:, j * P:(j + 1) * P], rhs=wkv_sb[:],
                            start=True, stop=True)
                    nc.scalar.activation(
                        out=kv_sb[:, c0:c0 + csz, :],
                        in_=pkv[:, :csz * 256].rearrange(
                            "p (b c) -> p b c", c=256),
                        func=COPY)

                psS = psB.tile([P, 132], f32, tag="acc")
                for (j0, gsz) in groups:
                    pqe = psA.tile([P, NB * 256], f32, tag="pbig")
                    for jj in range(gsz):
                        j = j0 + jj
                        nc.tensor.matmul(
                            pqe[:, jj * P:(jj + 1) * P],
                            lhsT=oh[:, j * P:(j + 1) * P],
                            rhs=q_loc[:, u * D:(u + 1) * D],
                            start=True, stop=True)
                    # qk product and per-head reduce -> alpha [P, gsz*H] f32
                    qk = sb.tile([P, 8, P], f16, tag="qk")
                    nc.vector.tensor_tensor(
                        out=qk[:, :gsz, :],
                        in0=pqe[:, :gsz * P].rearrange("p (a c) -> p a c", c=P),
                        in1=kv_sb[:, j0:j0 + gsz, 0:128], op=MUL)
                    alpha = sb.tile([P, 8 * H], f16, tag="alpha")
                    with nc.allow_low_precision("32-term f16 dot, matches v1 tree"):
                        nc.vector.tensor_reduce(
                            out=alpha[:, :gsz * H],
                            in_=qk[:, :gsz, :].rearrange(
                                "p a (h e) -> p (a h) e", e=DH),
                            axis=AXX, op=ADD)
                    # exp on the scalar engine; head-dim broadcast on gpsimd
                    X = sb.tile([P, 8, 132], f16, tag="X")
                    nc.scalar.activation(
                        out=X[:, :gsz, 128:132],
                        in_=alpha[:, :gsz * H].rearrange("p (a h) -> p a h", h=H),
                        func=EXP)
                    Xa = sb.tile([P, 8, P], f16, tag="Xa")
                    nc.gpsimd.tensor_tensor(
                        out=Xa[:, :gsz, :].rearrange(
                            "p a (h e) -> p a h e", e=DH),
                        in0=X[:, :gsz, 128:132, None].to_broadcast(
                            [P, gsz, H, DH]),
                        in1=ones_sb[:, None, None, :].to_broadcast(
                            [P, gsz, H, DH]),
                        op=MUL)
                    nc.vector.tensor_tensor(
                        out=X[:, :gsz, 0:128],
                        in0=kv_sb[:, j0:j0 + gsz, 128:256],
                        in1=Xa[:, :gsz, :], op=MUL)
                    for jj in range(gsz):
                        j = j0 + jj
                        nc.tensor.matmul(
                            psS[:, 0:132],
                            lhsT=ohT[:, (j0 + jj) * P:(j0 + jj + 1) * P],
                            rhs=X[:, jj, 0:132],
                            start=(j == 0), stop=(j == KU - 1))
                dn = sb.tile([P, H], f32, tag="dn")
                nc.vector.tensor_scalar(out=dn[:], in0=psS[:, 128:132],
                                        scalar1=1e-16, scalar2=None, op0=ADD)
                rc = sb.tile([P, H], f32, tag="rc")
                nc.vector.reciprocal(out=rc[:], in_=dn[:])
                ot = sb.tile([P, D], f32, tag="ot")
                for h in range(H):
                    nc.scalar.activation(
                        out=ot[:, h * DH:(h + 1) * DH],
                        in_=psS[:, h * DH:(h + 1) * DH],
                        func=COPY, scale=rc[:, h:h + 1])
                of = sb.tile([P, D], f32, tag="of")
                nc.vector.tensor_tensor(
                    out=of[:], in0=ot[:], in1=s_loc[:, u * D:(u + 1) * D], op=ADD)
                nc.sync.dma_start(out=out_t[u * P:(u + 1) * P, :], in_=of[:])

    nc.finalize()
    return nc


def _run(inputs, trace=False):
    _ensure_hooks()
    from concourse.bass_utils import run_bass_kernel_spmd

    meta = _prep(**inputs)
    key = (meta['TPC'], meta['NT'], meta['KU'])
    if key not in _prog_cache:
        _prog_cache[key] = _build(*key)
    nc = _prog_cache[key]
    res = run_bass_kernel_spmd(nc, meta['in_maps'],
                               core_ids=list(range(NCORES)), trace=trace)
    outs = [res.results[c]["out"] for c in range(NCORES)]
    out_perm = np.concatenate(outs, axis=0)
    out = out_perm[meta['node_slot']].astype(np.float32)
    return out, res


def kernel(**inputs) -> np.ndarray:
    out, _ = _run(inputs, trace=False)
    return out
